# revision 9
# baseline (speedup 1.0000x reference)
"""MinGRU block kernel for Trainium2 (Bass/Tile), SPMD over 8 NeuronCores.

Problem: B=8, S=2048, D=1024, F=3072 (nn_MinGRUBlock).
Sharding: data-parallel over batch (one batch row per core); weights replicated.

Per-core dataflow (all compute in "T layout": feature on partitions, time on free):
  phase 1 (mixer, s-chunks of 256):
    load x chunk [s,d] -> PE-transpose -> xT [d,s]
    rmsnorm row-scales r computed via squares + PE ones-reduce + sqrt/recip
    r broadcast across partitions via K=1 PE matmul
    g/v/d projections: fp16 matmuls (1 cyc/row on PE), fp32 PSUM accumulate
    sigmoid/tanh on ACT directly from PSUM (bias fused)
    h_t = a_t*h_{t-1} + x_t via DVE tensor_tensor_scan (fp32 state), chained
    across chunks with a carry column
    out1 = x + h; out1 and normalized out1n bounced to DRAM scratch
  phase 2 (FFN): stream W_gate/W_up once, z = silu(gate)*up in fp16 (12MB SBUF),
    then W_out matmuls + residual, PE-transpose back to [s,d], DMA out.
"""

import os
import sys
from contextlib import ExitStack

import numpy as np

for _p in ("/opt/trn_rl_repo", "/root/.axon_site/_ro/trn_rl_repo"):
    if os.path.isdir(_p) and _p not in sys.path:
        sys.path.insert(0, _p)

import concourse.bass as bass
import concourse.tile as tile
from concourse import bacc, mybir
from concourse.bass_utils import run_bass_kernel_spmd

F32 = mybir.dt.float32
F16 = mybir.dt.float16
AF = mybir.ActivationFunctionType
OP = mybir.AluOpType

B, S, D, F = 8, 2048, 1024, 3072
EPS = 1e-6
KD = D // 128          # 8 d-ptiles
MF2 = 2 * F // 128     # 48 f-ptiles (gate|up)
MFO = F // 128         # 24 f-ptiles
MD = D // 128          # 8 d-ptiles (output)

CH1 = 256              # phase-1 s-chunk
NCH1 = S // CH1
CH2 = 512              # phase-2 s-chunk
NCH2 = S // CH2
NST1 = CH1 // 128      # s-tiles per phase-1 chunk


def build_program():
    nc = bacc.Bacc("TRN2", target_bir_lowering=False, debug=False)

    x_d = nc.dram_tensor("x", [S, D], F32, kind="ExternalInput").ap()
    wmix_d = nc.dram_tensor("w_mix", [3 * MD, 128, KD, 128], F16, kind="ExternalInput").ap()
    bmix_d = nc.dram_tensor("b_mix", [128, 3 * MD], F32, kind="ExternalInput").ap()
    wgu_d = nc.dram_tensor("w_gu", [MF2, 128, KD, 128], F16, kind="ExternalInput").ap()
    wout_d = nc.dram_tensor("w_out", [MD, 128, MFO, 128], F16, kind="ExternalInput").ap()
    ident_d = nc.dram_tensor("ident", [128, 128], F32, kind="ExternalInput").ap()
    out_d = nc.dram_tensor("out", [S, D], F32, kind="ExternalOutput").ap()

    with tile.TileContext(nc) as tc, ExitStack() as top:
        # ---------- persistent tiles ----------
        cpool = top.enter_context(tc.tile_pool(name="consts", bufs=1))
        ident = cpool.tile([128, 128], F32)
        nc.sync.dma_start(ident[:], ident_d[:])
        ones_col = cpool.tile([128, 1], F16)
        nc.vector.memset(ones_col[:], 1.0)
        ones_row = cpool.tile([1, 128], F32)
        nc.vector.memset(ones_row[:], 1.0)
        zero128 = cpool.tile([128, 1], F32)
        nc.vector.memset(zero128[:], 0.0)
        eps1 = cpool.tile([1, 1], F32)
        nc.vector.memset(eps1[:], EPS)
        bmix = cpool.tile([128, 3 * MD], F32)
        nc.sync.dma_start(bmix[:], bmix_d[:])

        # DRAM scratch (tile-tracked so phase-2 reads order after phase-1 writes)
        dpool = top.enter_context(tc.tile_pool(name="dscratch", bufs=1, space="DRAM"))
        sc1 = dpool.tile([KD, 128, S], F32)      # out1 in T layout
        sc1n = dpool.tile([KD, 128, S], F16)     # normalized out1 in T layout
        sc1_p = sc1.rearrange("k p s -> p k s")
        sc1n_p = sc1n.rearrange("k p s -> p k s")

        carry_pool = top.enter_context(tc.tile_pool(name="carry", bufs=1))
        carry = carry_pool.tile([128, KD], F32)

        # ---------- phase 1: mixer ----------
        with ExitStack() as ph1:
            wpool = ph1.enter_context(tc.tile_pool(name="wmix", bufs=1))
            wmix = wpool.tile([128, 3 * MD, KD, 128], F16)
            nc.sync.dma_start(wmix[:], wmix_d.rearrange("m p k j -> p m k j"))

            p_nat = ph1.enter_context(tc.tile_pool(name="xnat", bufs=4))
            p_xT = ph1.enter_context(tc.tile_pool(name="xT", bufs=2))
            p_16 = ph1.enter_context(tc.tile_pool(name="f16bufs", bufs=2))
            p_32 = ph1.enter_context(tc.tile_pool(name="f32bufs", bufs=2))
            p_row = ph1.enter_context(tc.tile_pool(name="rows", bufs=2))
            ps_tp = ph1.enter_context(tc.tile_pool(name="tp_ps", bufs=1, space="PSUM"))
            ps_mm = ph1.enter_context(tc.tile_pool(name="mm_ps", bufs=2, space="PSUM"))
            ps_ss = ph1.enter_context(tc.tile_pool(name="ss_ps", bufs=1, space="PSUM"))
            ps_bc = ph1.enter_context(tc.tile_pool(name="bc_ps", bufs=2, space="PSUM"))

            def norm_row_scale(src, tag):
                """src: [128, KD, CH1] f32 SBUF -> bc_ps [128, CH1] PSUM with
                per-column 1/rms(src column)."""
                sq = p_16.tile([128, KD, CH1], F16, tag="sq")
                nc.scalar.activation(sq[:], src[:], AF.Square, bias=zero128[:])
                ss = ps_ss.tile([1, CH1], F32, tag="ss")
                for kt in range(KD):
                    nc.tensor.matmul(ss[:], ones_col[:], sq[:, kt],
                                     start=(kt == 0), stop=(kt == KD - 1))
                srow = p_row.tile([1, CH1], F32, tag=f"srow{tag}")
                nc.scalar.activation(srow[:], ss[:], AF.Sqrt, bias=eps1[:],
                                     scale=1.0 / D)
                rrow = p_row.tile([1, CH1], F32, tag=f"rrow{tag}")
                nc.vector.reciprocal(rrow[:], srow[:])
                bc = ps_bc.tile([128, CH1], F32, tag="bc")
                nc.tensor.matmul(bc[:], ones_row[:], rrow[:])
                return bc

            for c in range(NCH1):
                s0 = c * CH1
                # load + transpose x chunk
                xT = p_xT.tile([128, KD, CH1], F32, tag="xT")
                nats = []
                for st in range(NST1):
                    xn_t = p_nat.tile([128, D], F32, tag="xnat")
                    nc.sync.dma_start(xn_t[:], x_d[s0 + st * 128: s0 + (st + 1) * 128, :])
                    nats.append(xn_t)
                for kt in range(KD):
                    tp = ps_tp.tile([128, CH1], F32, tag="tp")
                    for st in range(NST1):
                        nc.tensor.transpose(tp[:, st * 128:(st + 1) * 128],
                                            nats[st][:, kt * 128:(kt + 1) * 128],
                                            ident[:])
                    nc.vector.tensor_copy(xT[:, kt], tp[:])

                # rms scale for mixer + normalized input (fp16)
                bc1 = norm_row_scale(xT, "1")
                xnT = p_16.tile([128, KD, CH1], F16, tag="xnT")
                for kt in range(KD):
                    nc.vector.tensor_tensor(xnT[:, kt], xT[:, kt], bc1[:], OP.mult)

                # projections g/v/d
                sig_g = p_16.tile([128, KD, CH1], F16, tag="sig_g")
                tanh_v = p_16.tile([128, KD, CH1], F16, tag="tanh_v")
                sig_d = p_16.tile([128, KD, CH1], F16, tag="sig_d")
                for proj, (dst, fn) in enumerate(
                        ((sig_g, AF.Sigmoid), (tanh_v, AF.Tanh), (sig_d, AF.Sigmoid))):
                    for half in range(2):
                        ps = ps_mm.tile([128, 4, CH1], F32, tag="mm")
                        for mi in range(4):
                            mt = proj * MD + half * 4 + mi
                            for kt in range(KD):
                                nc.tensor.matmul(ps[:, mi], wmix[:, mt, kt], xnT[:, kt],
                                                 start=(kt == 0), stop=(kt == KD - 1))
                        for mi in range(4):
                            mt = proj * MD + half * 4 + mi
                            nc.scalar.activation(dst[:, half * 4 + mi], ps[:, mi], fn,
                                                 bias=bmix[:, mt:mt + 1])

                # scan inputs
                xs = p_16.tile([128, KD, CH1], F16, tag="xs")
                nc.vector.tensor_tensor(xs[:], sig_g[:], tanh_v[:], OP.mult)
                a_t = p_16.tile([128, KD, CH1], F16, tag="a_t")
                nc.vector.tensor_scalar(a_t[:], sig_d[:], 0.998, 0.001, OP.mult, OP.add)

                # the scan itself (chained across chunks via carry)
                hT = p_32.tile([128, KD, CH1], F32, tag="hT")
                for kt in range(KD):
                    init = 0.0 if c == 0 else carry[:, kt:kt + 1]
                    nc.vector.tensor_tensor_scan(hT[:, kt], a_t[:, kt], xs[:, kt],
                                                 init, OP.mult, OP.add)
                    nc.vector.tensor_copy(carry[:, kt:kt + 1], hT[:, kt, CH1 - 1:CH1])

                # residual, second norm, bounce to DRAM
                out1 = p_32.tile([128, KD, CH1], F32, tag="out1")
                nc.vector.tensor_tensor(out1[:], xT[:], hT[:], OP.add)
                bc2 = norm_row_scale(out1, "2")
                out1n = p_16.tile([128, KD, CH1], F16, tag="out1n")
                for kt in range(KD):
                    nc.vector.tensor_tensor(out1n[:, kt], out1[:, kt], bc2[:], OP.mult)
                nc.sync.dma_start(sc1_p[:, :, s0:s0 + CH1], out1[:])
                nc.sync.dma_start(sc1n_p[:, :, s0:s0 + CH1], out1n[:])

        # ---------- phase 2: FFN ----------
        with ExitStack() as ph2:
            zpool = ph2.enter_context(tc.tile_pool(name="zbuf", bufs=1))
            z = zpool.tile([128, MFO, S], F16)

            # 2a: gate/up + z
            with ExitStack() as ph2a:
                o1n_pool = ph2a.enter_context(tc.tile_pool(name="o1n", bufs=1))
                o1n = o1n_pool.tile([128, KD, S], F16)
                nc.sync.dma_start(o1n[:], sc1n_p[:])
                p_wgu = ph2a.enter_context(tc.tile_pool(name="wgu", bufs=4))
                p_gu = ph2a.enter_context(tc.tile_pool(name="gu16", bufs=3))
                ps_gu = ph2a.enter_context(tc.tile_pool(name="gu_ps", bufs=4, space="PSUM"))
                for mg in range(MFO):
                    wg = p_wgu.tile([128, KD, 128], F16, tag="wgu")
                    nc.sync.dma_start(wg[:], wgu_d[mg])
                    wu = p_wgu.tile([128, KD, 128], F16, tag="wgu")
                    nc.sync.dma_start(wu[:], wgu_d[MFO + mg])
                    for sc in range(NCH2):
                        sl = slice(sc * CH2, (sc + 1) * CH2)
                        gps = ps_gu.tile([128, CH2], F32, tag="gups")
                        for kt in range(KD):
                            nc.tensor.matmul(gps[:], wg[:, kt], o1n[:, kt, sl],
                                             start=(kt == 0), stop=(kt == KD - 1))
                        ups = ps_gu.tile([128, CH2], F32, tag="gups")
                        for kt in range(KD):
                            nc.tensor.matmul(ups[:], wu[:, kt], o1n[:, kt, sl],
                                             start=(kt == 0), stop=(kt == KD - 1))
                        sig = p_gu.tile([128, CH2], F16, tag="sig")
                        nc.scalar.activation(sig[:], gps[:], AF.Sigmoid, bias=zero128[:])
                        gate = p_gu.tile([128, CH2], F16, tag="gate")
                        nc.vector.tensor_tensor(gate[:], gps[:], sig[:], OP.mult)
                        up = p_gu.tile([128, CH2], F16, tag="up")
                        nc.scalar.copy(up[:], ups[:])
                        nc.vector.tensor_tensor(z[:, mg, sl], gate[:], up[:], OP.mult)

            # 2b: W_out + residual + transpose out
            with ExitStack() as ph2b:
                p_wo = ph2b.enter_context(tc.tile_pool(name="wout", bufs=1))
                wout = p_wo.tile([128, MD, MFO, 128], F16)
                nc.sync.dma_start(wout[:], wout_d.rearrange("m p k j -> p m k j"))
                p_o1c = ph2b.enter_context(tc.tile_pool(name="o1c", bufs=3))
                p_oT = ph2b.enter_context(tc.tile_pool(name="outT", bufs=MD + 1))
                p_onat = ph2b.enter_context(tc.tile_pool(name="onat", bufs=3))
                ps_y = ph2b.enter_context(tc.tile_pool(name="y_ps", bufs=2, space="PSUM"))
                ps_t2 = ph2b.enter_context(tc.tile_pool(name="t2_ps", bufs=2, space="PSUM"))
                for sc in range(NCH2):
                    sl = slice(sc * CH2, (sc + 1) * CH2)
                    outTs = []
                    for mo in range(MD):
                        yps = ps_y.tile([128, CH2], F32, tag="yps")
                        for kt in range(MFO):
                            nc.tensor.matmul(yps[:], wout[:, mo, kt], z[:, kt, sl],
                                             start=(kt == 0), stop=(kt == MFO - 1))
                        o1c = p_o1c.tile([128, CH2], F32, tag="o1c")
                        nc.sync.dma_start(o1c[:], sc1[mo, :, sl])
                        oT = p_oT.tile([128, CH2], F32, tag="oT")
                        nc.vector.tensor_tensor(oT[:], yps[:], o1c[:], OP.add)
                        outTs.append(oT)
                    for q in range(CH2 // 128):
                        onat = p_onat.tile([128, D], F32, tag="onat")
                        for h in range(2):
                            t2 = ps_t2.tile([128, 512], F32, tag="t2")
                            for j in range(4):
                                nc.tensor.transpose(
                                    t2[:, j * 128:(j + 1) * 128],
                                    outTs[4 * h + j][:, q * 128:(q + 1) * 128],
                                    ident[:])
                            nc.scalar.copy(onat[:, h * 512:(h + 1) * 512], t2[:])
                        srow0 = sc * CH2 + q * 128
                        nc.sync.dma_start(out_d[srow0:srow0 + 128, :], onat[:])

    nc.compile()
    return nc


_NC = None


def _get_nc():
    global _NC
    if _NC is None:
        _NC = build_program()
    return _NC


def _prep_weights(inputs):
    w1 = np.asarray(inputs["rms_mix_w"], np.float32)
    w2 = np.asarray(inputs["rms_ffn_w"], np.float32)
    Wg = np.asarray(inputs["Wg"], np.float32) * w1[None, :]
    Wv = np.asarray(inputs["Wv"], np.float32) * w1[None, :]
    Wd = np.asarray(inputs["Wd"], np.float32) * w1[None, :]
    Wcat = np.concatenate([Wg, Wv, Wd], axis=0)            # [3D, D]
    w_mix = np.ascontiguousarray(
        Wcat.T.reshape(KD, 128, 3 * MD, 128).transpose(2, 1, 0, 3)).astype(np.float16)
    bcat = np.concatenate([np.asarray(inputs["bg"], np.float32),
                           np.asarray(inputs["bv"], np.float32),
                           np.asarray(inputs["bd"], np.float32)])
    b_mix = np.ascontiguousarray(bcat.reshape(3 * MD, 128).T).astype(np.float32)
    Wgate = np.asarray(inputs["W_gate"], np.float32) * w2[None, :]
    Wup = np.asarray(inputs["W_up"], np.float32) * w2[None, :]
    Wcat2 = np.concatenate([Wgate, Wup], axis=0)           # [2F, D]
    w_gu = np.ascontiguousarray(
        Wcat2.T.reshape(KD, 128, MF2, 128).transpose(2, 1, 0, 3)).astype(np.float16)
    WoT = np.asarray(inputs["W_out"], np.float32).T        # [F, D]
    w_out = np.ascontiguousarray(
        WoT.reshape(MFO, 128, MD, 128).transpose(2, 1, 0, 3)).astype(np.float16)
    return {
        "w_mix": w_mix, "b_mix": b_mix, "w_gu": w_gu, "w_out": w_out,
        "ident": np.eye(128, dtype=np.float32),
    }


def run(inputs, trace=False, **kw):
    x = np.asarray(inputs["x"], np.float32)
    shared = _prep_weights(inputs)
    in_maps = [dict(shared, x=np.ascontiguousarray(x[b])) for b in range(B)]
    res = run_bass_kernel_spmd(_get_nc(), in_maps, list(range(B)), trace=trace, **kw)
    out = np.stack([np.asarray(res.results[b]["out"], np.float32) for b in range(B)])
    return out, res


def kernel(**inputs) -> np.ndarray:
    out, _ = run(inputs)
    return out


# revision 12
# speedup vs baseline: 1.1805x; 1.1805x over previous
"""MinGRU block kernel for Trainium2 (Bass/Tile), SPMD over 8 NeuronCores.

Problem: B=8, S=2048, D=1024, F=3072 (nn_MinGRUBlock).
Sharding: data-parallel over batch (one batch row per core); weights replicated.

Per-core dataflow (all compute in "T layout": feature on partitions, time on free):
  phase 1 (mixer, s-chunks of 256):
    load x chunk [s,d] -> PE-transpose -> xT [d,s]
    rmsnorm row-scales r computed via squares + PE ones-reduce + sqrt/recip
    r broadcast across partitions via K=1 PE matmul
    g/v/d projections: fp16 matmuls (1 cyc/row on PE), fp32 PSUM accumulate
    sigmoid/tanh on ACT directly from PSUM (bias fused)
    h_t = a_t*h_{t-1} + x_t via DVE tensor_tensor_scan (fp32 state), chained
    across chunks with a carry column
    out1 = x + h; out1 and normalized out1n bounced to DRAM scratch
  phase 2 (FFN): stream W_gate/W_up once, z = silu(gate)*up in fp16 (12MB SBUF),
    then W_out matmuls + residual, PE-transpose back to [s,d], DMA out.
"""

import os
import sys
from contextlib import ExitStack

import numpy as np

for _p in ("/opt/trn_rl_repo", "/root/.axon_site/_ro/trn_rl_repo"):
    if os.path.isdir(_p) and _p not in sys.path:
        sys.path.insert(0, _p)

import concourse.bass as bass
import concourse.tile as tile
from concourse import bacc, mybir
from concourse.bass_utils import run_bass_kernel_spmd

F32 = mybir.dt.float32
F16 = mybir.dt.float16
AF = mybir.ActivationFunctionType
OP = mybir.AluOpType

B, S, D, F = 8, 2048, 1024, 3072
EPS = 1e-6
KD = D // 128          # 8 d-ptiles
MF2 = 2 * F // 128     # 48 f-ptiles (gate|up)
MFO = F // 128         # 24 f-ptiles
MD = D // 128          # 8 d-ptiles (output)

CH1 = 256              # phase-1 s-chunk
NCH1 = S // CH1
CH2 = 512              # phase-2 s-chunk
NCH2 = S // CH2
NST1 = CH1 // 128      # s-tiles per phase-1 chunk


def build_program():
    nc = bacc.Bacc("TRN2", target_bir_lowering=False, debug=False)

    x_d = nc.dram_tensor("x", [S, D], F32, kind="ExternalInput").ap()
    wmix_d = nc.dram_tensor("w_mix", [3 * MD, 128, KD, 128], F16, kind="ExternalInput").ap()
    bmix_d = nc.dram_tensor("b_mix", [128, 3 * MD], F32, kind="ExternalInput").ap()
    wgu_d = nc.dram_tensor("w_gu", [MF2, 128, KD, 128], F16, kind="ExternalInput").ap()
    wout_d = nc.dram_tensor("w_out", [MD, 128, MFO, 128], F16, kind="ExternalInput").ap()
    ident_d = nc.dram_tensor("ident", [128, 128], F32, kind="ExternalInput").ap()
    out_d = nc.dram_tensor("out", [S, D], F32, kind="ExternalOutput").ap()

    with tile.TileContext(nc) as tc, ExitStack() as top:
        # ---------- persistent tiles ----------
        cpool = top.enter_context(tc.tile_pool(name="consts", bufs=1))
        ident = cpool.tile([128, 128], F32)
        nc.sync.dma_start(ident[:], ident_d[:])
        ones_col = cpool.tile([128, 1], F16)
        nc.vector.memset(ones_col[:], 1.0)
        ones_row = cpool.tile([1, 128], F32)
        nc.vector.memset(ones_row[:], 1.0)
        zero128 = cpool.tile([128, 1], F32)
        nc.vector.memset(zero128[:], 0.0)
        eps1 = cpool.tile([1, 1], F32)
        nc.vector.memset(eps1[:], EPS)
        bmix = cpool.tile([128, 3 * MD], F32)
        nc.sync.dma_start(bmix[:], bmix_d[:])

        # DRAM scratch (tile-tracked so phase-2 reads order after phase-1 writes)
        dpool = top.enter_context(tc.tile_pool(name="dscratch", bufs=1, space="DRAM"))
        sc1 = dpool.tile([KD, 128, S], F32)      # out1 in T layout
        sc1n = dpool.tile([KD, 128, S], F16)     # normalized out1 in T layout
        sc1_p = sc1.rearrange("k p s -> p k s")
        sc1n_p = sc1n.rearrange("k p s -> p k s")

        carry_pool = top.enter_context(tc.tile_pool(name="carry", bufs=1))
        carry = carry_pool.tile([128, KD], F32)

        # ---------- phase 1: mixer (software-pipelined over chunks) ----------
        with ExitStack() as ph1:
            wpool = ph1.enter_context(tc.tile_pool(name="wmix", bufs=1))
            wmix = wpool.tile([128, 3 * MD, KD, 128], F16)
            wmix_dp = wmix_d.rearrange("m p k j -> p m k j")

            p_nat = ph1.enter_context(tc.tile_pool(name="xnat", bufs=4))
            p_xT = ph1.enter_context(tc.tile_pool(name="xT", bufs=3))
            p_16 = ph1.enter_context(tc.tile_pool(name="f16bufs", bufs=2))
            p_sq = ph1.enter_context(tc.tile_pool(name="sqbufs", bufs=3))
            p_32 = ph1.enter_context(tc.tile_pool(name="f32bufs", bufs=2))
            p_row = ph1.enter_context(tc.tile_pool(name="rows", bufs=2))
            ps_tp = ph1.enter_context(tc.tile_pool(name="tp_ps", bufs=2, space="PSUM"))
            ps_mm = ph1.enter_context(tc.tile_pool(name="mm_ps", bufs=3, space="PSUM"))
            ps_ss = ph1.enter_context(tc.tile_pool(name="ss_ps", bufs=1, space="PSUM"))
            ps_bc = ph1.enter_context(tc.tile_pool(name="bc_ps", bufs=2, space="PSUM"))

            st_front = {}   # c -> (xT, rrow1)
            st_bc1 = {}     # c -> bc1 psum tile
            st_body = {}    # c -> (sig_g, tanh_v, sig_d)
            st_back = {}    # c -> (out1, rrow2)

            def front(c):
                """load + transpose x chunk, squares, norm1 reduce/sqrt/recip."""
                s0 = c * CH1
                xT = p_xT.tile([128, KD, CH1], F32, tag="xT", name=f"xT{c}")
                nats = []
                for st in range(NST1):
                    xn_t = p_nat.tile([128, D], F32, tag="xnat", name=f"xnat{c}_{st}")
                    nc.sync.dma_start(xn_t[:], x_d[s0 + st * 128: s0 + (st + 1) * 128, :])
                    nats.append(xn_t)
                for kt in range(KD):
                    tp = ps_tp.tile([128, CH1], F32, tag="tp", name=f"tp{c}_{kt}")
                    for st in range(NST1):
                        nc.tensor.transpose(tp[:, st * 128:(st + 1) * 128],
                                            nats[st][:, kt * 128:(kt + 1) * 128],
                                            ident[:])
                    nc.scalar.copy(xT[:, kt], tp[:])
                sq = p_sq.tile([128, KD, CH1], F16, tag="sq", name=f"sq1_{c}")
                nc.scalar.activation(sq[:], xT[:], AF.Square, bias=zero128[:])
                ss = ps_ss.tile([1, CH1], F32, tag="ss", name=f"ss1_{c}")
                for kt in range(KD):
                    nc.tensor.matmul(ss[:], ones_col[:], sq[:, kt],
                                     start=(kt == 0), stop=(kt == KD - 1))
                srow = p_row.tile([1, CH1], F32, tag="srow1", name=f"srow1_{c}")
                nc.scalar.activation(srow[:], ss[:], AF.Sqrt, bias=eps1[:], scale=1.0 / D)
                rrow = p_row.tile([1, CH1], F32, tag="rrow1", name=f"rrow1_{c}")
                nc.vector.reciprocal(rrow[:], srow[:])
                st_front[c] = (xT, rrow)

            def bcast1(c):
                rrow = st_front[c][1]
                bc = ps_bc.tile([128, CH1], F32, tag="bc", name=f"bc1_{c}")
                nc.tensor.matmul(bc[:], ones_row[:], rrow[:])
                st_bc1[c] = bc

            def body(c):
                """normalized input + g/v/d projections + activations."""
                xT = st_front[c][0]
                bc1 = st_bc1[c]
                xnT = p_16.tile([128, KD, CH1], F16, tag="xnT", name=f"xnT{c}")
                for kt in range(KD):
                    nc.vector.tensor_tensor(xnT[:, kt], xT[:, kt], bc1[:], OP.mult)
                sig_g = p_16.tile([128, KD, CH1], F16, tag="sig_g", name=f"sg{c}")
                tanh_v = p_16.tile([128, KD, CH1], F16, tag="tanh_v", name=f"tv{c}")
                sig_d = p_16.tile([128, KD, CH1], F16, tag="sig_d", name=f"sd{c}")
                for proj, (dst, fn) in enumerate(
                        ((sig_g, AF.Sigmoid), (tanh_v, AF.Tanh), (sig_d, AF.Sigmoid))):
                    for half in range(4):
                        ps = ps_mm.tile([128, 2, CH1], F32, tag="mm",
                                        name=f"mm{c}_{proj}_{half}")
                        for mi in range(2):
                            mt = proj * MD + half * 2 + mi
                            for kt in range(KD):
                                nc.tensor.matmul(ps[:, mi], wmix[:, mt, kt], xnT[:, kt],
                                                 start=(kt == 0), stop=(kt == KD - 1))
                        for mi in range(2):
                            mt = proj * MD + half * 2 + mi
                            nc.scalar.activation(dst[:, half * 2 + mi], ps[:, mi], fn,
                                                 bias=bmix[:, mt:mt + 1])
                st_body[c] = (sig_g, tanh_v, sig_d)

            def back_a(c):
                """scan inputs, scan, residual, norm2 squares+reduce."""
                sig_g, tanh_v, sig_d = st_body[c]
                xT = st_front[c][0]
                xs = p_16.tile([128, KD, CH1], F16, tag="xs", name=f"xs{c}")
                nc.vector.tensor_tensor(xs[:], sig_g[:], tanh_v[:], OP.mult)
                a_t = p_16.tile([128, KD, CH1], F16, tag="a_t", name=f"a{c}")
                nc.vector.tensor_scalar(a_t[:], sig_d[:], 0.998, 0.001, OP.mult, OP.add)
                hT = p_32.tile([128, KD, CH1], F32, tag="hT", name=f"hT{c}")
                for kt in range(KD):
                    init = 0.0 if c == 0 else carry[:, kt:kt + 1]
                    nc.vector.tensor_tensor_scan(hT[:, kt], a_t[:, kt], xs[:, kt],
                                                 init, OP.mult, OP.add)
                    nc.vector.tensor_copy(carry[:, kt:kt + 1], hT[:, kt, CH1 - 1:CH1])
                out1 = p_32.tile([128, KD, CH1], F32, tag="out1", name=f"o1_{c}")
                nc.vector.tensor_tensor(out1[:], xT[:], hT[:], OP.add)
                sq = p_sq.tile([128, KD, CH1], F16, tag="sq", name=f"sq2_{c}")
                nc.scalar.activation(sq[:], out1[:], AF.Square, bias=zero128[:])
                ss = ps_ss.tile([1, CH1], F32, tag="ss", name=f"ss2_{c}")
                for kt in range(KD):
                    nc.tensor.matmul(ss[:], ones_col[:], sq[:, kt],
                                     start=(kt == 0), stop=(kt == KD - 1))
                srow = p_row.tile([1, CH1], F32, tag="srow2", name=f"srow2_{c}")
                nc.scalar.activation(srow[:], ss[:], AF.Sqrt, bias=eps1[:], scale=1.0 / D)
                rrow = p_row.tile([1, CH1], F32, tag="rrow2", name=f"rrow2_{c}")
                nc.vector.reciprocal(rrow[:], srow[:])
                st_back[c] = (out1, rrow)

            def back_b(c):
                """norm2 broadcast, out1n, DMA bounce."""
                s0 = c * CH1
                out1, rrow = st_back[c]
                bc = ps_bc.tile([128, CH1], F32, tag="bc", name=f"bc2_{c}")
                nc.tensor.matmul(bc[:], ones_row[:], rrow[:])
                out1n = p_16.tile([128, KD, CH1], F16, tag="out1n", name=f"o1n{c}")
                for kt in range(KD):
                    nc.vector.tensor_tensor(out1n[:, kt], out1[:, kt], bc[:], OP.mult)
                nc.sync.dma_start(sc1_p[:, :, s0:s0 + CH1], out1[:])
                nc.sync.dma_start(sc1n_p[:, :, s0:s0 + CH1], out1n[:])

            # pipelined emission: PE stream per cycle is
            #   [T(c+1) R1(c+1)] [MM(c)] [R2(c-1)] [B1(c+1)] [B2(c-1)]
            front(0)
            # mixer weights after first chunk's transposes are queued
            for proj in range(3):
                nc.sync.dma_start(wmix[:, proj * MD:(proj + 1) * MD],
                                  wmix_dp[:, proj * MD:(proj + 1) * MD])
            bcast1(0)
            for c in range(NCH1):
                if c + 1 < NCH1:
                    front(c + 1)
                body(c)
                if c >= 1:
                    back_a(c - 1)
                if c + 1 < NCH1:
                    bcast1(c + 1)
                if c >= 1:
                    back_b(c - 1)
            back_a(NCH1 - 1)
            back_b(NCH1 - 1)

        # ---------- phase 2: FFN ----------
        with ExitStack() as ph2:
            zpool = ph2.enter_context(tc.tile_pool(name="zbuf", bufs=1))
            z = zpool.tile([128, MFO, S], F16)
            p_wo = ph2.enter_context(tc.tile_pool(name="wout", bufs=1))
            wout = p_wo.tile([128, MD, MFO, 128], F16)
            nc.sync.dma_start(wout[:], wout_d.rearrange("m p k j -> p m k j"))

            # 2a: gate/up + z
            with ExitStack() as ph2a:
                o1n_pool = ph2a.enter_context(tc.tile_pool(name="o1n", bufs=1))
                o1n = o1n_pool.tile([128, KD, S], F16)
                for c in range(NCH1):
                    sl0 = slice(c * CH1, (c + 1) * CH1)
                    nc.sync.dma_start(o1n[:, :, sl0], sc1n_p[:, :, sl0])
                p_wgu = ph2a.enter_context(tc.tile_pool(name="wgu", bufs=3))
                p_gu = ph2a.enter_context(tc.tile_pool(name="gu16", bufs=2))
                ps_gu = ph2a.enter_context(tc.tile_pool(name="gu_ps", bufs=4, space="PSUM"))
                for mg in range(MFO):
                    wg = p_wgu.tile([128, KD, 128], F16, tag="wgu")
                    nc.sync.dma_start(wg[:], wgu_d[mg])
                    wu = p_wgu.tile([128, KD, 128], F16, tag="wgu")
                    nc.sync.dma_start(wu[:], wgu_d[MFO + mg])
                    for sc in range(NCH2):
                        sl = slice(sc * CH2, (sc + 1) * CH2)
                        gps = ps_gu.tile([128, CH2], F32, tag="gups")
                        for kt in range(KD):
                            nc.tensor.matmul(gps[:], wg[:, kt], o1n[:, kt, sl],
                                             start=(kt == 0), stop=(kt == KD - 1))
                        ups = ps_gu.tile([128, CH2], F32, tag="gups")
                        for kt in range(KD):
                            nc.tensor.matmul(ups[:], wu[:, kt], o1n[:, kt, sl],
                                             start=(kt == 0), stop=(kt == KD - 1))
                        sig = p_gu.tile([128, CH2], F16, tag="sig")
                        nc.scalar.activation(sig[:], gps[:], AF.Sigmoid, bias=zero128[:])
                        gate = p_gu.tile([128, CH2], F16, tag="gate")
                        nc.vector.tensor_tensor(gate[:], gps[:], sig[:], OP.mult)
                        up = p_gu.tile([128, CH2], F16, tag="up")
                        nc.scalar.copy(up[:], ups[:])
                        nc.vector.tensor_tensor(z[:, mg, sl], gate[:], up[:], OP.mult)

            # 2b: W_out + residual + transpose out
            with ExitStack() as ph2b:
                p_o1c = ph2b.enter_context(tc.tile_pool(name="o1c", bufs=3))
                p_oT = ph2b.enter_context(tc.tile_pool(name="outT", bufs=MD + 1))
                p_onat = ph2b.enter_context(tc.tile_pool(name="onat", bufs=3))
                ps_y = ph2b.enter_context(tc.tile_pool(name="y_ps", bufs=2, space="PSUM"))
                ps_t2 = ph2b.enter_context(tc.tile_pool(name="t2_ps", bufs=2, space="PSUM"))
                for sc in range(NCH2):
                    sl = slice(sc * CH2, (sc + 1) * CH2)
                    outTs = []
                    for mo in range(MD):
                        yps = ps_y.tile([128, CH2], F32, tag="yps")
                        for kt in range(MFO):
                            nc.tensor.matmul(yps[:], wout[:, mo, kt], z[:, kt, sl],
                                             start=(kt == 0), stop=(kt == MFO - 1))
                        o1c = p_o1c.tile([128, CH2], F32, tag="o1c")
                        nc.sync.dma_start(o1c[:], sc1[mo, :, sl])
                        oT = p_oT.tile([128, CH2], F32, tag="oT")
                        nc.vector.tensor_tensor(oT[:], yps[:], o1c[:], OP.add)
                        outTs.append(oT)
                    for q in range(CH2 // 128):
                        onat = p_onat.tile([128, D], F32, tag="onat")
                        for h in range(2):
                            t2 = ps_t2.tile([128, 512], F32, tag="t2")
                            for j in range(4):
                                nc.tensor.transpose(
                                    t2[:, j * 128:(j + 1) * 128],
                                    outTs[4 * h + j][:, q * 128:(q + 1) * 128],
                                    ident[:])
                            nc.scalar.copy(onat[:, h * 512:(h + 1) * 512], t2[:])
                        srow0 = sc * CH2 + q * 128
                        nc.sync.dma_start(out_d[srow0:srow0 + 128, :], onat[:])

    nc.compile()
    return nc


_NC = None


def _get_nc():
    global _NC
    if _NC is None:
        _NC = build_program()
    return _NC


def _prep_weights(inputs):
    w1 = np.asarray(inputs["rms_mix_w"], np.float32)
    w2 = np.asarray(inputs["rms_ffn_w"], np.float32)
    Wg = np.asarray(inputs["Wg"], np.float32) * w1[None, :]
    Wv = np.asarray(inputs["Wv"], np.float32) * w1[None, :]
    Wd = np.asarray(inputs["Wd"], np.float32) * w1[None, :]
    Wcat = np.concatenate([Wg, Wv, Wd], axis=0)            # [3D, D]
    w_mix = np.ascontiguousarray(
        Wcat.T.reshape(KD, 128, 3 * MD, 128).transpose(2, 1, 0, 3)).astype(np.float16)
    bcat = np.concatenate([np.asarray(inputs["bg"], np.float32),
                           np.asarray(inputs["bv"], np.float32),
                           np.asarray(inputs["bd"], np.float32)])
    b_mix = np.ascontiguousarray(bcat.reshape(3 * MD, 128).T).astype(np.float32)
    Wgate = np.asarray(inputs["W_gate"], np.float32) * w2[None, :]
    Wup = np.asarray(inputs["W_up"], np.float32) * w2[None, :]
    Wcat2 = np.concatenate([Wgate, Wup], axis=0)           # [2F, D]
    w_gu = np.ascontiguousarray(
        Wcat2.T.reshape(KD, 128, MF2, 128).transpose(2, 1, 0, 3)).astype(np.float16)
    WoT = np.asarray(inputs["W_out"], np.float32).T        # [F, D]
    w_out = np.ascontiguousarray(
        WoT.reshape(MFO, 128, MD, 128).transpose(2, 1, 0, 3)).astype(np.float16)
    return {
        "w_mix": w_mix, "b_mix": b_mix, "w_gu": w_gu, "w_out": w_out,
        "ident": np.eye(128, dtype=np.float32),
    }


def run(inputs, trace=False, **kw):
    x = np.asarray(inputs["x"], np.float32)
    shared = _prep_weights(inputs)
    in_maps = [dict(shared, x=np.ascontiguousarray(x[b])) for b in range(B)]
    res = run_bass_kernel_spmd(_get_nc(), in_maps, list(range(B)), trace=trace, **kw)
    out = np.stack([np.asarray(res.results[b]["out"], np.float32) for b in range(B)])
    return out, res


def kernel(**inputs) -> np.ndarray:
    out, _ = run(inputs)
    return out


# revision 16
# speedup vs baseline: 1.2063x; 1.0218x over previous
"""MinGRU block kernel for Trainium2 (Bass/Tile), SPMD over 8 NeuronCores.

Problem: B=8, S=2048, D=1024, F=3072 (nn_MinGRUBlock).
Sharding: data-parallel over batch (one batch row per core); weights replicated.

Per-core dataflow (all compute in "T layout": feature on partitions, time on free):
  phase 1 (mixer, s-chunks of 256):
    load x chunk [s,d] -> PE-transpose -> xT [d,s]
    rmsnorm row-scales r computed via squares + PE ones-reduce + sqrt/recip
    r broadcast across partitions via K=1 PE matmul
    g/v/d projections: fp16 matmuls (1 cyc/row on PE), fp32 PSUM accumulate
    sigmoid/tanh on ACT directly from PSUM (bias fused)
    h_t = a_t*h_{t-1} + x_t via DVE tensor_tensor_scan (fp32 state), chained
    across chunks with a carry column
    out1 = x + h; out1 and normalized out1n bounced to DRAM scratch
  phase 2 (FFN): stream W_gate/W_up once, z = silu(gate)*up in fp16 (12MB SBUF),
    then W_out matmuls + residual, PE-transpose back to [s,d], DMA out.
"""

import os
import sys
from contextlib import ExitStack

import numpy as np

for _p in ("/opt/trn_rl_repo", "/root/.axon_site/_ro/trn_rl_repo"):
    if os.path.isdir(_p) and _p not in sys.path:
        sys.path.insert(0, _p)

import concourse.bass as bass
import concourse.tile as tile
from concourse import bacc, mybir
from concourse.bass_utils import run_bass_kernel_spmd

F32 = mybir.dt.float32
F16 = mybir.dt.float16
AF = mybir.ActivationFunctionType
OP = mybir.AluOpType

B, S, D, F = 8, 2048, 1024, 3072
EPS = 1e-6
KD = D // 128          # 8 d-ptiles
MF2 = 2 * F // 128     # 48 f-ptiles (gate|up)
MFO = F // 128         # 24 f-ptiles
MD = D // 128          # 8 d-ptiles (output)

CH1 = 256              # phase-1 s-chunk
NCH1 = S // CH1
CH2 = 512              # phase-2 s-chunk
NCH2 = S // CH2
NST1 = CH1 // 128      # s-tiles per phase-1 chunk


def build_program():
    nc = bacc.Bacc("TRN2", target_bir_lowering=False, debug=False)

    x_d = nc.dram_tensor("x", [S, D], F32, kind="ExternalInput").ap()
    wmix_d = nc.dram_tensor("w_mix", [3 * MD, 128, KD, 128], F16, kind="ExternalInput").ap()
    bmix_d = nc.dram_tensor("b_mix", [128, 3 * MD], F32, kind="ExternalInput").ap()
    wgu_d = nc.dram_tensor("w_gu", [MF2, 128, KD, 128], F16, kind="ExternalInput").ap()
    wout_d = nc.dram_tensor("w_out", [MD, 128, MFO, 128], F16, kind="ExternalInput").ap()
    ident_d = nc.dram_tensor("ident", [128, 128], F32, kind="ExternalInput").ap()
    out_d = nc.dram_tensor("out", [S, D], F32, kind="ExternalOutput").ap()

    with tile.TileContext(nc) as tc, ExitStack() as top:
        # ---------- persistent tiles ----------
        cpool = top.enter_context(tc.tile_pool(name="consts", bufs=1))
        ident = cpool.tile([128, 128], F32)
        nc.sync.dma_start(ident[:], ident_d[:])
        ones_col = cpool.tile([128, 1], F16)
        nc.vector.memset(ones_col[:], 1.0)
        ones_row = cpool.tile([1, 128], F32)
        nc.vector.memset(ones_row[:], 1.0)
        zero128 = cpool.tile([128, 1], F32)
        nc.vector.memset(zero128[:], 0.0)
        eps1 = cpool.tile([1, 1], F32)
        nc.vector.memset(eps1[:], EPS)
        bmix = cpool.tile([128, 3 * MD], F32)
        nc.sync.dma_start(bmix[:], bmix_d[:])

        # DRAM scratch (tile-tracked so phase-2 reads order after phase-1
        # writes). One tile per phase-2 s-chunk so a phase-2 load only
        # depends on the phase-1 chunks that actually wrote it.
        dpool = top.enter_context(tc.tile_pool(name="dscratch", bufs=1, space="DRAM"))
        sc1_t = [dpool.tile([KD, 128, CH2], F32, name=f"sc1_{i}") for i in range(NCH2)]
        sc1n_t = [dpool.tile([KD, 128, CH2], F16, name=f"sc1n_{i}") for i in range(NCH2)]
        sc1_p = [t.rearrange("k p s -> p k s") for t in sc1_t]
        sc1n_p = [t.rearrange("k p s -> p k s") for t in sc1n_t]

        carry_pool = top.enter_context(tc.tile_pool(name="carry", bufs=1))
        carry = carry_pool.tile([128, KD], F32)

        # ---------- phase 1: mixer (software-pipelined over chunks) ----------
        with ExitStack() as ph1:
            wpool = ph1.enter_context(tc.tile_pool(name="wmix", bufs=1))
            wmix = wpool.tile([128, 3 * MD, KD, 128], F16)
            wmix_dp = wmix_d.rearrange("m p k j -> p m k j")

            p_nat = ph1.enter_context(tc.tile_pool(name="xnat", bufs=4))
            p_xT = ph1.enter_context(tc.tile_pool(name="xT", bufs=3))
            p_16 = ph1.enter_context(tc.tile_pool(name="f16bufs", bufs=2))
            p_sq = ph1.enter_context(tc.tile_pool(name="sqbufs", bufs=3))
            p_32 = ph1.enter_context(tc.tile_pool(name="f32bufs", bufs=2))
            p_row = ph1.enter_context(tc.tile_pool(name="rows", bufs=2))
            ps_tp = ph1.enter_context(tc.tile_pool(name="tp_ps", bufs=2, space="PSUM"))
            ps_mm = ph1.enter_context(tc.tile_pool(name="mm_ps", bufs=3, space="PSUM"))
            ps_ss = ph1.enter_context(tc.tile_pool(name="ss_ps", bufs=1, space="PSUM"))
            ps_bc = ph1.enter_context(tc.tile_pool(name="bc_ps", bufs=2, space="PSUM"))

            st_front = {}   # c -> (xT, rrow1)
            st_bc1 = {}     # c -> bc1 psum tile
            st_body = {}    # c -> (sig_g, tanh_v, sig_d)
            st_back = {}    # c -> (out1, rrow2)

            def front(c):
                """load + transpose x chunk, squares, norm1 reduce/sqrt/recip."""
                s0 = c * CH1
                xT = p_xT.tile([128, KD, CH1], F32, tag="xT", name=f"xT{c}")
                nats = []
                for st in range(NST1):
                    xn_t = p_nat.tile([128, D], F32, tag="xnat", name=f"xnat{c}_{st}")
                    nc.sync.dma_start(xn_t[:], x_d[s0 + st * 128: s0 + (st + 1) * 128, :])
                    nats.append(xn_t)
                for kt in range(KD):
                    tp = ps_tp.tile([128, CH1], F32, tag="tp", name=f"tp{c}_{kt}")
                    for st in range(NST1):
                        nc.tensor.transpose(tp[:, st * 128:(st + 1) * 128],
                                            nats[st][:, kt * 128:(kt + 1) * 128],
                                            ident[:])
                    nc.scalar.copy(xT[:, kt], tp[:])
                sq = p_sq.tile([128, KD, CH1], F16, tag="sq", name=f"sq1_{c}")
                nc.scalar.activation(sq[:], xT[:], AF.Square, bias=zero128[:])
                ss = ps_ss.tile([1, CH1], F32, tag="ss", name=f"ss1_{c}")
                for kt in range(KD):
                    nc.tensor.matmul(ss[:], ones_col[:], sq[:, kt],
                                     start=(kt == 0), stop=(kt == KD - 1))
                srow = p_row.tile([1, CH1], F32, tag="srow1", name=f"srow1_{c}")
                nc.scalar.activation(srow[:], ss[:], AF.Sqrt, bias=eps1[:], scale=1.0 / D)
                rrow = p_row.tile([1, CH1], F32, tag="rrow1", name=f"rrow1_{c}")
                nc.vector.reciprocal(rrow[:], srow[:])
                st_front[c] = (xT, rrow)

            def bcast1(c):
                rrow = st_front[c][1]
                bc = ps_bc.tile([128, CH1], F32, tag="bc", name=f"bc1_{c}")
                nc.tensor.matmul(bc[:], ones_row[:], rrow[:])
                st_bc1[c] = bc

            def body(c):
                """normalized input + g/v/d projections + activations."""
                xT = st_front[c][0]
                bc1 = st_bc1[c]
                xnT = p_16.tile([128, KD, CH1], F16, tag="xnT", name=f"xnT{c}")
                for kt in range(KD):
                    nc.vector.tensor_tensor(xnT[:, kt], xT[:, kt], bc1[:], OP.mult)
                sig_g = p_16.tile([128, KD, CH1], F16, tag="sig_g", name=f"sg{c}")
                tanh_v = p_16.tile([128, KD, CH1], F16, tag="tanh_v", name=f"tv{c}")
                sig_d = p_16.tile([128, KD, CH1], F16, tag="sig_d", name=f"sd{c}")
                for proj, (dst, fn) in enumerate(
                        ((sig_g, AF.Sigmoid), (tanh_v, AF.Tanh), (sig_d, AF.Sigmoid))):
                    for half in range(4):
                        ps = ps_mm.tile([128, 2, CH1], F32, tag="mm",
                                        name=f"mm{c}_{proj}_{half}")
                        for mi in range(2):
                            mt = proj * MD + half * 2 + mi
                            for kt in range(KD):
                                nc.tensor.matmul(ps[:, mi], wmix[:, mt, kt], xnT[:, kt],
                                                 start=(kt == 0), stop=(kt == KD - 1))
                        for mi in range(2):
                            mt = proj * MD + half * 2 + mi
                            nc.scalar.activation(dst[:, half * 2 + mi], ps[:, mi], fn,
                                                 bias=bmix[:, mt:mt + 1])
                st_body[c] = (sig_g, tanh_v, sig_d)

            def back_a(c):
                """scan inputs, scan, residual, norm2 squares+reduce."""
                sig_g, tanh_v, sig_d = st_body[c]
                xT = st_front[c][0]
                xs = p_16.tile([128, KD, CH1], F16, tag="xs", name=f"xs{c}")
                nc.vector.tensor_tensor(xs[:], sig_g[:], tanh_v[:], OP.mult)
                a_t = p_16.tile([128, KD, CH1], F16, tag="a_t", name=f"a{c}")
                nc.vector.tensor_scalar(a_t[:], sig_d[:], 0.998, 0.001, OP.mult, OP.add)
                hT = p_32.tile([128, KD, CH1], F32, tag="hT", name=f"hT{c}")
                for kt in range(KD):
                    init = 0.0 if c == 0 else carry[:, kt:kt + 1]
                    nc.vector.tensor_tensor_scan(hT[:, kt], a_t[:, kt], xs[:, kt],
                                                 init, OP.mult, OP.add)
                    nc.vector.tensor_copy(carry[:, kt:kt + 1], hT[:, kt, CH1 - 1:CH1])
                out1 = p_32.tile([128, KD, CH1], F32, tag="out1", name=f"o1_{c}")
                nc.vector.tensor_tensor(out1[:], xT[:], hT[:], OP.add)
                sq = p_sq.tile([128, KD, CH1], F16, tag="sq", name=f"sq2_{c}")
                nc.scalar.activation(sq[:], out1[:], AF.Square, bias=zero128[:])
                ss = ps_ss.tile([1, CH1], F32, tag="ss", name=f"ss2_{c}")
                for kt in range(KD):
                    nc.tensor.matmul(ss[:], ones_col[:], sq[:, kt],
                                     start=(kt == 0), stop=(kt == KD - 1))
                srow = p_row.tile([1, CH1], F32, tag="srow2", name=f"srow2_{c}")
                nc.scalar.activation(srow[:], ss[:], AF.Sqrt, bias=eps1[:], scale=1.0 / D)
                rrow = p_row.tile([1, CH1], F32, tag="rrow2", name=f"rrow2_{c}")
                nc.vector.reciprocal(rrow[:], srow[:])
                st_back[c] = (out1, rrow)

            def back_b(c):
                """norm2 broadcast, out1n, DMA bounce."""
                s0 = c * CH1
                out1, rrow = st_back[c]
                bc = ps_bc.tile([128, CH1], F32, tag="bc", name=f"bc2_{c}")
                nc.tensor.matmul(bc[:], ones_row[:], rrow[:])
                out1n = p_16.tile([128, KD, CH1], F16, tag="out1n", name=f"o1n{c}")
                for kt in range(KD):
                    nc.vector.tensor_tensor(out1n[:, kt], out1[:, kt], bc[:], OP.mult)
                sc, off = divmod(s0, CH2)
                nc.sync.dma_start(sc1_p[sc][:, :, off:off + CH1], out1[:])
                nc.sync.dma_start(sc1n_p[sc][:, :, off:off + CH1], out1n[:])

            # pipelined emission: PE stream per cycle is
            #   [T(c+1) R1(c+1)] [MM(c)] [R2(c-1)] [B1(c+1)] [B2(c-1)]
            front(0)
            # mixer weights after first chunk's transposes are queued
            for proj in range(3):
                nc.sync.dma_start(wmix[:, proj * MD:(proj + 1) * MD],
                                  wmix_dp[:, proj * MD:(proj + 1) * MD])
            bcast1(0)
            for c in range(NCH1):
                if c + 1 < NCH1:
                    front(c + 1)
                body(c)
                if c >= 1:
                    back_a(c - 1)
                if c + 1 < NCH1:
                    bcast1(c + 1)
                if c >= 1:
                    back_b(c - 1)
            back_a(NCH1 - 1)
            back_b(NCH1 - 1)

        # ---------- phase 2: FFN ----------
        with ExitStack() as ph2:
            zpool = ph2.enter_context(tc.tile_pool(name="zbuf", bufs=1))
            z = zpool.tile([128, MFO, S], F16)
            p_wo = ph2.enter_context(tc.tile_pool(name="wout", bufs=1))
            wout = p_wo.tile([128, MD, MFO, 128], F16)
            nc.sync.dma_start(wout[:], wout_d.rearrange("m p k j -> p m k j"))

            # 2a: gate/up + z
            with ExitStack() as ph2a:
                o1n_pool = ph2a.enter_context(tc.tile_pool(name="o1n", bufs=1))
                o1n = o1n_pool.tile([128, KD, S], F16)
                for sc in range(NCH2):
                    nc.sync.dma_start(o1n[:, :, sc * CH2:(sc + 1) * CH2], sc1n_p[sc][:])
                p_wgu = ph2a.enter_context(tc.tile_pool(name="wgu", bufs=3))
                p_gu = ph2a.enter_context(tc.tile_pool(name="gu16", bufs=2))
                ps_gu = ph2a.enter_context(tc.tile_pool(name="gu_ps", bufs=4, space="PSUM"))
                for mg in range(MFO):
                    wg = p_wgu.tile([128, KD, 128], F16, tag="wgu")
                    nc.sync.dma_start(wg[:], wgu_d[mg])
                    wu = p_wgu.tile([128, KD, 128], F16, tag="wgu")
                    nc.sync.dma_start(wu[:], wgu_d[MFO + mg])
                    for sc in range(NCH2):
                        sl = slice(sc * CH2, (sc + 1) * CH2)
                        gps = ps_gu.tile([128, CH2], F32, tag="gups")
                        for kt in range(KD):
                            nc.tensor.matmul(gps[:], wg[:, kt], o1n[:, kt, sl],
                                             start=(kt == 0), stop=(kt == KD - 1))
                        ups = ps_gu.tile([128, CH2], F32, tag="gups")
                        for kt in range(KD):
                            nc.tensor.matmul(ups[:], wu[:, kt], o1n[:, kt, sl],
                                             start=(kt == 0), stop=(kt == KD - 1))
                        sig = p_gu.tile([128, CH2], F16, tag="sig")
                        nc.scalar.activation(sig[:], gps[:], AF.Sigmoid, bias=zero128[:])
                        gate = p_gu.tile([128, CH2], F16, tag="gate")
                        nc.vector.tensor_tensor(gate[:], gps[:], sig[:], OP.mult)
                        up = p_gu.tile([128, CH2], F16, tag="up")
                        nc.scalar.copy(up[:], ups[:])
                        nc.vector.tensor_tensor(z[:, mg, sl], gate[:], up[:], OP.mult)

            # 2b: W_out + residual + transpose out
            with ExitStack() as ph2b:
                p_o1c = ph2b.enter_context(tc.tile_pool(name="o1c", bufs=3))
                p_oT = ph2b.enter_context(tc.tile_pool(name="outT", bufs=MD + 1))
                p_onat = ph2b.enter_context(tc.tile_pool(name="onat", bufs=3))
                ps_y = ph2b.enter_context(tc.tile_pool(name="y_ps", bufs=2, space="PSUM"))
                ps_t2 = ph2b.enter_context(tc.tile_pool(name="t2_ps", bufs=2, space="PSUM"))
                for sc in range(NCH2):
                    sl = slice(sc * CH2, (sc + 1) * CH2)
                    outTs = []
                    for mo in range(MD):
                        yps = ps_y.tile([128, CH2], F32, tag="yps")
                        for kt in range(MFO):
                            nc.tensor.matmul(yps[:], wout[:, mo, kt], z[:, kt, sl],
                                             start=(kt == 0), stop=(kt == MFO - 1))
                        o1c = p_o1c.tile([128, CH2], F32, tag="o1c")
                        nc.sync.dma_start(o1c[:], sc1_t[sc][mo])
                        oT = p_oT.tile([128, CH2], F32, tag="oT")
                        nc.vector.tensor_tensor(oT[:], yps[:], o1c[:], OP.add)
                        outTs.append(oT)
                    for q in range(CH2 // 128):
                        onat = p_onat.tile([128, D], F32, tag="onat")
                        for h in range(2):
                            t2 = ps_t2.tile([128, 512], F32, tag="t2")
                            for j in range(4):
                                nc.tensor.transpose(
                                    t2[:, j * 128:(j + 1) * 128],
                                    outTs[4 * h + j][:, q * 128:(q + 1) * 128],
                                    ident[:])
                            nc.scalar.copy(onat[:, h * 512:(h + 1) * 512], t2[:])
                        srow0 = sc * CH2 + q * 128
                        nc.sync.dma_start(out_d[srow0:srow0 + 128, :], onat[:])

    nc.compile()
    return nc


_NC = None


def _get_nc():
    global _NC
    if _NC is None:
        _NC = build_program()
    return _NC


def _prep_weights(inputs):
    w1 = np.asarray(inputs["rms_mix_w"], np.float32)
    w2 = np.asarray(inputs["rms_ffn_w"], np.float32)
    Wg = np.asarray(inputs["Wg"], np.float32) * w1[None, :]
    Wv = np.asarray(inputs["Wv"], np.float32) * w1[None, :]
    Wd = np.asarray(inputs["Wd"], np.float32) * w1[None, :]
    Wcat = np.concatenate([Wg, Wv, Wd], axis=0)            # [3D, D]
    w_mix = np.ascontiguousarray(
        Wcat.T.reshape(KD, 128, 3 * MD, 128).transpose(2, 1, 0, 3)).astype(np.float16)
    bcat = np.concatenate([np.asarray(inputs["bg"], np.float32),
                           np.asarray(inputs["bv"], np.float32),
                           np.asarray(inputs["bd"], np.float32)])
    b_mix = np.ascontiguousarray(bcat.reshape(3 * MD, 128).T).astype(np.float32)
    Wgate = np.asarray(inputs["W_gate"], np.float32) * w2[None, :]
    Wup = np.asarray(inputs["W_up"], np.float32) * w2[None, :]
    Wcat2 = np.concatenate([Wgate, Wup], axis=0)           # [2F, D]
    w_gu = np.ascontiguousarray(
        Wcat2.T.reshape(KD, 128, MF2, 128).transpose(2, 1, 0, 3)).astype(np.float16)
    WoT = np.asarray(inputs["W_out"], np.float32).T        # [F, D]
    w_out = np.ascontiguousarray(
        WoT.reshape(MFO, 128, MD, 128).transpose(2, 1, 0, 3)).astype(np.float16)
    return {
        "w_mix": w_mix, "b_mix": b_mix, "w_gu": w_gu, "w_out": w_out,
        "ident": np.eye(128, dtype=np.float32),
    }


def run(inputs, trace=False, **kw):
    x = np.asarray(inputs["x"], np.float32)
    shared = _prep_weights(inputs)
    in_maps = [dict(shared, x=np.ascontiguousarray(x[b])) for b in range(B)]
    res = run_bass_kernel_spmd(_get_nc(), in_maps, list(range(B)), trace=trace, **kw)
    out = np.stack([np.asarray(res.results[b]["out"], np.float32) for b in range(B)])
    return out, res


def kernel(**inputs) -> np.ndarray:
    out, _ = run(inputs)
    return out


# revision 19
# speedup vs baseline: 1.2098x; 1.0029x over previous
"""MinGRU block kernel for Trainium2 (Bass/Tile), SPMD over 8 NeuronCores.

Problem: B=8, S=2048, D=1024, F=3072 (nn_MinGRUBlock).
Sharding: data-parallel over batch (one batch row per core); weights replicated.

Per-core dataflow (all compute in "T layout": feature on partitions, time on free):
  phase 1 (mixer, s-chunks of 256):
    load x chunk [s,d] -> PE-transpose -> xT [d,s]
    rmsnorm row-scales r computed via squares + PE ones-reduce + sqrt/recip
    r broadcast across partitions via K=1 PE matmul
    g/v/d projections: fp16 matmuls (1 cyc/row on PE), fp32 PSUM accumulate
    sigmoid/tanh on ACT directly from PSUM (bias fused)
    h_t = a_t*h_{t-1} + x_t via DVE tensor_tensor_scan (fp32 state), chained
    across chunks with a carry column
    out1 = x + h; out1 and normalized out1n bounced to DRAM scratch
  phase 2 (FFN): stream W_gate/W_up once, z = silu(gate)*up in fp16 (12MB SBUF),
    then W_out matmuls + residual, PE-transpose back to [s,d], DMA out.
"""

import os
import sys
from contextlib import ExitStack

import numpy as np

for _p in ("/opt/trn_rl_repo", "/root/.axon_site/_ro/trn_rl_repo"):
    if os.path.isdir(_p) and _p not in sys.path:
        sys.path.insert(0, _p)

import concourse.bass as bass
import concourse.tile as tile
from concourse import bacc, mybir
from concourse.bass_utils import run_bass_kernel_spmd

F32 = mybir.dt.float32
F16 = mybir.dt.float16
AF = mybir.ActivationFunctionType
OP = mybir.AluOpType

B, S, D, F = 8, 2048, 1024, 3072
EPS = 1e-6
KD = D // 128          # 8 d-ptiles
MF2 = 2 * F // 128     # 48 f-ptiles (gate|up)
MFO = F // 128         # 24 f-ptiles
MD = D // 128          # 8 d-ptiles (output)

CH1 = 256              # phase-1 s-chunk
NCH1 = S // CH1
CH2 = 512              # phase-2 s-chunk
NCH2 = S // CH2
NST1 = CH1 // 128      # s-tiles per phase-1 chunk


def build_program():
    nc = bacc.Bacc("TRN2", target_bir_lowering=False, debug=False)

    x_d = nc.dram_tensor("x", [S, D], F32, kind="ExternalInput").ap()
    wmix_d = nc.dram_tensor("w_mix", [3 * MD, 128, KD, 128], F16, kind="ExternalInput").ap()
    bmix_d = nc.dram_tensor("b_mix", [128, 3 * MD], F32, kind="ExternalInput").ap()
    wgu_d = nc.dram_tensor("w_gu", [MF2, 128, KD, 128], F16, kind="ExternalInput").ap()
    wout_d = nc.dram_tensor("w_out", [MD, 128, MFO, 128], F16, kind="ExternalInput").ap()
    ident_d = nc.dram_tensor("ident", [128, 128], F32, kind="ExternalInput").ap()
    out_d = nc.dram_tensor("out", [S, D], F32, kind="ExternalOutput").ap()

    with tile.TileContext(nc) as tc, ExitStack() as top:
        # ---------- persistent tiles ----------
        cpool = top.enter_context(tc.tile_pool(name="consts", bufs=1))
        ident = cpool.tile([128, 128], F32)
        nc.sync.dma_start(ident[:], ident_d[:])
        ones_col = cpool.tile([128, 1], F16)
        nc.vector.memset(ones_col[:], 1.0)
        ones_row = cpool.tile([1, 128], F32)
        nc.vector.memset(ones_row[:], 1.0)
        zero128 = cpool.tile([128, 1], F32)
        nc.vector.memset(zero128[:], 0.0)
        eps1 = cpool.tile([1, 1], F32)
        nc.vector.memset(eps1[:], EPS)
        bmix = cpool.tile([128, 3 * MD], F32)
        nc.sync.dma_start(bmix[:], bmix_d[:])

        # DRAM scratch (tile-tracked so phase-2 reads order after phase-1
        # writes). One tile per phase-2 s-chunk so a phase-2 load only
        # depends on the phase-1 chunks that actually wrote it.
        dpool = top.enter_context(tc.tile_pool(name="dscratch", bufs=1, space="DRAM"))
        sc1_t = [dpool.tile([KD, 128, CH2], F32, name=f"sc1_{i}") for i in range(NCH2)]
        sc1n_t = [dpool.tile([KD, 128, CH2], F16, name=f"sc1n_{i}") for i in range(NCH2)]
        sc1_p = [t.rearrange("k p s -> p k s") for t in sc1_t]
        sc1n_p = [t.rearrange("k p s -> p k s") for t in sc1n_t]

        carry_pool = top.enter_context(tc.tile_pool(name="carry", bufs=1))
        carry = carry_pool.tile([128, KD], F32)

        # ---------- phase 1: mixer (software-pipelined over chunks) ----------
        with ExitStack() as ph1:
            wpool = ph1.enter_context(tc.tile_pool(name="wmix", bufs=1))
            wmix = wpool.tile([128, 3 * MD, KD, 128], F16)
            wmix_dp = wmix_d.rearrange("m p k j -> p m k j")

            p_nat = ph1.enter_context(tc.tile_pool(name="xnat", bufs=4))
            p_xT = ph1.enter_context(tc.tile_pool(name="xT", bufs=3))
            p_16 = ph1.enter_context(tc.tile_pool(name="f16bufs", bufs=2))
            p_sq = ph1.enter_context(tc.tile_pool(name="sqbufs", bufs=3))
            p_32 = ph1.enter_context(tc.tile_pool(name="f32bufs", bufs=2))
            p_row = ph1.enter_context(tc.tile_pool(name="rows", bufs=2))
            ps_tp = ph1.enter_context(tc.tile_pool(name="tp_ps", bufs=2, space="PSUM"))
            ps_mm = ph1.enter_context(tc.tile_pool(name="mm_ps", bufs=3, space="PSUM"))
            ps_ss = ph1.enter_context(tc.tile_pool(name="ss_ps", bufs=1, space="PSUM"))
            ps_bc = ph1.enter_context(tc.tile_pool(name="bc_ps", bufs=2, space="PSUM"))

            st_front = {}   # c -> (xT, rrow1)
            st_bc1 = {}     # c -> bc1 psum tile
            st_body = {}    # c -> (sig_g, tanh_v, sig_d)
            st_back = {}    # c -> (out1, rrow2)

            st_sq1 = {}

            def front_t(c):
                """load + transpose x chunk, squares (ACT)."""
                s0 = c * CH1
                xT = p_xT.tile([128, KD, CH1], F32, tag="xT", name=f"xT{c}")
                sq = p_sq.tile([128, KD, CH1], F16, tag="sq", name=f"sq1_{c}")
                nats = []
                for st in range(NST1):
                    xn_t = p_nat.tile([128, D], F32, tag="xnat", name=f"xnat{c}_{st}")
                    nc.sync.dma_start(xn_t[:], x_d[s0 + st * 128: s0 + (st + 1) * 128, :])
                    nats.append(xn_t)
                for kt in range(KD):
                    tp = ps_tp.tile([128, CH1], F32, tag="tp", name=f"tp{c}_{kt}")
                    for st in range(NST1):
                        nc.tensor.transpose(tp[:, st * 128:(st + 1) * 128],
                                            nats[st][:, kt * 128:(kt + 1) * 128],
                                            ident[:])
                    nc.scalar.copy(xT[:, kt], tp[:])
                    nc.scalar.activation(sq[:, kt], xT[:, kt], AF.Square,
                                         bias=zero128[:])
                st_front[c] = (xT, None)
                st_sq1[c] = sq

            def front_r(c):
                """norm1 reduce (PE) + sqrt/recip."""
                sq = st_sq1[c]
                ss = ps_ss.tile([1, CH1], F32, tag="ss", name=f"ss1_{c}")
                for kt in range(KD):
                    nc.tensor.matmul(ss[:], ones_col[:], sq[:, kt],
                                     start=(kt == 0), stop=(kt == KD - 1))
                srow = p_row.tile([1, CH1], F32, tag="srow1", name=f"srow1_{c}")
                nc.scalar.activation(srow[:], ss[:], AF.Sqrt, bias=eps1[:], scale=1.0 / D)
                rrow = p_row.tile([1, CH1], F32, tag="rrow1", name=f"rrow1_{c}")
                nc.vector.reciprocal(rrow[:], srow[:])
                st_front[c] = (st_front[c][0], rrow)

            def bcast1(c):
                rrow = st_front[c][1]
                bc = ps_bc.tile([128, CH1], F32, tag="bc", name=f"bc1_{c}")
                nc.tensor.matmul(bc[:], ones_row[:], rrow[:])
                st_bc1[c] = bc

            def body(c):
                """normalized input + g/v/d projections + activations."""
                xT = st_front[c][0]
                bc1 = st_bc1[c]
                xnT = p_16.tile([128, KD, CH1], F16, tag="xnT", name=f"xnT{c}")
                for kt in range(KD):
                    nc.vector.tensor_tensor(xnT[:, kt], xT[:, kt], bc1[:], OP.mult)
                sig_g = p_16.tile([128, KD, CH1], F16, tag="sig_g", name=f"sg{c}")
                tanh_v = p_16.tile([128, KD, CH1], F16, tag="tanh_v", name=f"tv{c}")
                sig_d = p_16.tile([128, KD, CH1], F16, tag="sig_d", name=f"sd{c}")
                for proj, (dst, fn) in enumerate(
                        ((sig_g, AF.Sigmoid), (tanh_v, AF.Tanh), (sig_d, AF.Sigmoid))):
                    for half in range(4):
                        ps = ps_mm.tile([128, 2, CH1], F32, tag="mm",
                                        name=f"mm{c}_{proj}_{half}")
                        for mi in range(2):
                            mt = proj * MD + half * 2 + mi
                            for kt in range(KD):
                                nc.tensor.matmul(ps[:, mi], wmix[:, mt, kt], xnT[:, kt],
                                                 start=(kt == 0), stop=(kt == KD - 1))
                        for mi in range(2):
                            mt = proj * MD + half * 2 + mi
                            nc.scalar.activation(dst[:, half * 2 + mi], ps[:, mi], fn,
                                                 bias=bmix[:, mt:mt + 1])
                st_body[c] = (sig_g, tanh_v, sig_d)

            def back_a(c):
                """scan inputs, scan, residual, norm2 squares+reduce.

                Per-kt pipeline so the norm2 PE reduce starts while later
                kt rows are still scanning on DVE."""
                sig_g, tanh_v, sig_d = st_body[c]
                xT = st_front[c][0]
                xs = p_16.tile([128, KD, CH1], F16, tag="xs", name=f"xs{c}")
                nc.vector.tensor_tensor(xs[:], sig_g[:], tanh_v[:], OP.mult)
                a_t = p_16.tile([128, KD, CH1], F16, tag="a_t", name=f"a{c}")
                nc.vector.tensor_scalar(a_t[:], sig_d[:], 0.998, 0.001, OP.mult, OP.add)
                hT = p_32.tile([128, KD, CH1], F32, tag="hT", name=f"hT{c}")
                out1 = p_32.tile([128, KD, CH1], F32, tag="out1", name=f"o1_{c}")
                sq = p_sq.tile([128, KD, CH1], F16, tag="sq", name=f"sq2_{c}")
                ss = ps_ss.tile([1, CH1], F32, tag="ss", name=f"ss2_{c}")
                for kt in range(KD):
                    init = 0.0 if c == 0 else carry[:, kt:kt + 1]
                    nc.vector.tensor_tensor_scan(hT[:, kt], a_t[:, kt], xs[:, kt],
                                                 init, OP.mult, OP.add)
                    nc.vector.tensor_copy(carry[:, kt:kt + 1], hT[:, kt, CH1 - 1:CH1])
                    nc.vector.tensor_tensor(out1[:, kt], xT[:, kt], hT[:, kt], OP.add)
                    nc.scalar.activation(sq[:, kt], out1[:, kt], AF.Square,
                                         bias=zero128[:])
                    nc.tensor.matmul(ss[:], ones_col[:], sq[:, kt],
                                     start=(kt == 0), stop=(kt == KD - 1))
                srow = p_row.tile([1, CH1], F32, tag="srow2", name=f"srow2_{c}")
                nc.scalar.activation(srow[:], ss[:], AF.Sqrt, bias=eps1[:], scale=1.0 / D)
                rrow = p_row.tile([1, CH1], F32, tag="rrow2", name=f"rrow2_{c}")
                nc.vector.reciprocal(rrow[:], srow[:])
                st_back[c] = (out1, rrow)

            def back_b(c):
                """norm2 broadcast, out1n, DMA bounce."""
                s0 = c * CH1
                out1, rrow = st_back[c]
                bc = ps_bc.tile([128, CH1], F32, tag="bc", name=f"bc2_{c}")
                nc.tensor.matmul(bc[:], ones_row[:], rrow[:])
                out1n = p_16.tile([128, KD, CH1], F16, tag="out1n", name=f"o1n{c}")
                for kt in range(KD):
                    nc.vector.tensor_tensor(out1n[:, kt], out1[:, kt], bc[:], OP.mult)
                sc, off = divmod(s0, CH2)
                nc.sync.dma_start(sc1_p[sc][:, :, off:off + CH1], out1[:])
                nc.sync.dma_start(sc1n_p[sc][:, :, off:off + CH1], out1n[:])

            # pipelined emission; PE stream per cycle:
            #   [T(c+1)] [MM(c)] [R1(c+1)] [R2(c-1)] [B1(c+1)] [B2(c-1)]
            front_t(0)
            # mixer weights per e-ptile so the first projections start early
            for mt in range(3 * MD):
                nc.sync.dma_start(wmix[:, mt], wmix_dp[:, mt])
            front_r(0)
            bcast1(0)
            for c in range(NCH1):
                if c + 1 < NCH1:
                    front_t(c + 1)
                body(c)
                if c + 1 < NCH1:
                    front_r(c + 1)
                if c >= 1:
                    back_a(c - 1)
                if c + 1 < NCH1:
                    bcast1(c + 1)
                if c >= 1:
                    back_b(c - 1)
            back_a(NCH1 - 1)
            back_b(NCH1 - 1)

        # ---------- phase 2: FFN ----------
        with ExitStack() as ph2:
            zpool = ph2.enter_context(tc.tile_pool(name="zbuf", bufs=1))
            z = zpool.tile([128, MFO, S], F16)
            p_wo = ph2.enter_context(tc.tile_pool(name="wout", bufs=1))
            wout = p_wo.tile([128, MD, MFO, 128], F16)
            nc.sync.dma_start(wout[:], wout_d.rearrange("m p k j -> p m k j"))

            # 2a: gate/up + z
            with ExitStack() as ph2a:
                o1n_pool = ph2a.enter_context(tc.tile_pool(name="o1n", bufs=1))
                o1n = o1n_pool.tile([128, KD, S], F16)
                for sc in range(NCH2):
                    nc.sync.dma_start(o1n[:, :, sc * CH2:(sc + 1) * CH2], sc1n_p[sc][:])
                p_wgu = ph2a.enter_context(tc.tile_pool(name="wgu", bufs=3))
                p_gu = ph2a.enter_context(tc.tile_pool(name="gu16", bufs=2))
                ps_gu = ph2a.enter_context(tc.tile_pool(name="gu_ps", bufs=4, space="PSUM"))
                for mg in range(MFO):
                    wg = p_wgu.tile([128, KD, 128], F16, tag="wgu")
                    nc.sync.dma_start(wg[:], wgu_d[mg])
                    wu = p_wgu.tile([128, KD, 128], F16, tag="wgu")
                    nc.sync.dma_start(wu[:], wgu_d[MFO + mg])
                    for sc in range(NCH2):
                        sl = slice(sc * CH2, (sc + 1) * CH2)
                        gps = ps_gu.tile([128, CH2], F32, tag="gups")
                        for kt in range(KD):
                            nc.tensor.matmul(gps[:], wg[:, kt], o1n[:, kt, sl],
                                             start=(kt == 0), stop=(kt == KD - 1))
                        ups = ps_gu.tile([128, CH2], F32, tag="gups")
                        for kt in range(KD):
                            nc.tensor.matmul(ups[:], wu[:, kt], o1n[:, kt, sl],
                                             start=(kt == 0), stop=(kt == KD - 1))
                        sig = p_gu.tile([128, CH2], F16, tag="sig")
                        nc.scalar.activation(sig[:], gps[:], AF.Sigmoid, bias=zero128[:])
                        gate = p_gu.tile([128, CH2], F16, tag="gate")
                        nc.vector.tensor_tensor(gate[:], gps[:], sig[:], OP.mult)
                        up = p_gu.tile([128, CH2], F16, tag="up")
                        nc.scalar.copy(up[:], ups[:])
                        nc.vector.tensor_tensor(z[:, mg, sl], gate[:], up[:], OP.mult)

            # 2b: W_out + residual + transpose out
            with ExitStack() as ph2b:
                p_o1c = ph2b.enter_context(tc.tile_pool(name="o1c", bufs=3))
                p_oT = ph2b.enter_context(tc.tile_pool(name="outT", bufs=MD + 1))
                p_onat = ph2b.enter_context(tc.tile_pool(name="onat", bufs=3))
                ps_y = ph2b.enter_context(tc.tile_pool(name="y_ps", bufs=2, space="PSUM"))
                ps_t2 = ph2b.enter_context(tc.tile_pool(name="t2_ps", bufs=2, space="PSUM"))
                for sc in range(NCH2):
                    sl = slice(sc * CH2, (sc + 1) * CH2)
                    outTs = []
                    for mo in range(MD):
                        yps = ps_y.tile([128, CH2], F32, tag="yps")
                        for kt in range(MFO):
                            nc.tensor.matmul(yps[:], wout[:, mo, kt], z[:, kt, sl],
                                             start=(kt == 0), stop=(kt == MFO - 1))
                        o1c = p_o1c.tile([128, CH2], F32, tag="o1c")
                        nc.sync.dma_start(o1c[:], sc1_t[sc][mo])
                        oT = p_oT.tile([128, CH2], F32, tag="oT")
                        nc.vector.tensor_tensor(oT[:], yps[:], o1c[:], OP.add)
                        outTs.append(oT)
                    for q in range(CH2 // 128):
                        onat = p_onat.tile([128, D], F32, tag="onat")
                        for h in range(2):
                            t2 = ps_t2.tile([128, 512], F32, tag="t2")
                            for j in range(4):
                                nc.tensor.transpose(
                                    t2[:, j * 128:(j + 1) * 128],
                                    outTs[4 * h + j][:, q * 128:(q + 1) * 128],
                                    ident[:])
                            nc.scalar.copy(onat[:, h * 512:(h + 1) * 512], t2[:])
                        srow0 = sc * CH2 + q * 128
                        nc.sync.dma_start(out_d[srow0:srow0 + 128, :], onat[:])

    nc.compile()
    return nc


_NC = None


def _get_nc():
    global _NC
    if _NC is None:
        _NC = build_program()
    return _NC


def _prep_weights(inputs):
    w1 = np.asarray(inputs["rms_mix_w"], np.float32)
    w2 = np.asarray(inputs["rms_ffn_w"], np.float32)
    Wg = np.asarray(inputs["Wg"], np.float32) * w1[None, :]
    Wv = np.asarray(inputs["Wv"], np.float32) * w1[None, :]
    Wd = np.asarray(inputs["Wd"], np.float32) * w1[None, :]
    Wcat = np.concatenate([Wg, Wv, Wd], axis=0)            # [3D, D]
    w_mix = np.ascontiguousarray(
        Wcat.T.reshape(KD, 128, 3 * MD, 128).transpose(2, 1, 0, 3)).astype(np.float16)
    bcat = np.concatenate([np.asarray(inputs["bg"], np.float32),
                           np.asarray(inputs["bv"], np.float32),
                           np.asarray(inputs["bd"], np.float32)])
    b_mix = np.ascontiguousarray(bcat.reshape(3 * MD, 128).T).astype(np.float32)
    Wgate = np.asarray(inputs["W_gate"], np.float32) * w2[None, :]
    Wup = np.asarray(inputs["W_up"], np.float32) * w2[None, :]
    Wcat2 = np.concatenate([Wgate, Wup], axis=0)           # [2F, D]
    w_gu = np.ascontiguousarray(
        Wcat2.T.reshape(KD, 128, MF2, 128).transpose(2, 1, 0, 3)).astype(np.float16)
    WoT = np.asarray(inputs["W_out"], np.float32).T        # [F, D]
    w_out = np.ascontiguousarray(
        WoT.reshape(MFO, 128, MD, 128).transpose(2, 1, 0, 3)).astype(np.float16)
    return {
        "w_mix": w_mix, "b_mix": b_mix, "w_gu": w_gu, "w_out": w_out,
        "ident": np.eye(128, dtype=np.float32),
    }


def run(inputs, trace=False, **kw):
    x = np.asarray(inputs["x"], np.float32)
    shared = _prep_weights(inputs)
    in_maps = [dict(shared, x=np.ascontiguousarray(x[b])) for b in range(B)]
    res = run_bass_kernel_spmd(_get_nc(), in_maps, list(range(B)), trace=trace, **kw)
    out = np.stack([np.asarray(res.results[b]["out"], np.float32) for b in range(B)])
    return out, res


def kernel(**inputs) -> np.ndarray:
    out, _ = run(inputs)
    return out


# revision 24
# speedup vs baseline: 1.2612x; 1.0425x over previous
"""MinGRU block kernel for Trainium2 (Bass/Tile), SPMD over 8 NeuronCores.

Problem: B=8, S=2048, D=1024, F=3072 (nn_MinGRUBlock).
Sharding: data-parallel over batch (one batch row per core); weights replicated.

Per-core dataflow (all compute in "T layout": feature on partitions, time on free):
  phase 1 (mixer, s-chunks of 256):
    load x chunk [s,d] -> PE-transpose -> xT [d,s]
    rmsnorm row-scales r computed via squares + PE ones-reduce + sqrt/recip
    r broadcast across partitions via K=1 PE matmul
    g/v/d projections: fp16 matmuls (1 cyc/row on PE), fp32 PSUM accumulate
    sigmoid/tanh on ACT directly from PSUM (bias fused)
    h_t = a_t*h_{t-1} + x_t via DVE tensor_tensor_scan (fp32 state), chained
    across chunks with a carry column
    out1 = x + h; out1 and normalized out1n bounced to DRAM scratch
  phase 2 (FFN): stream W_gate/W_up once, z = silu(gate)*up in fp16 (12MB SBUF),
    then W_out matmuls + residual, PE-transpose back to [s,d], DMA out.
"""

import os
import sys
from contextlib import ExitStack

import numpy as np

for _p in ("/opt/trn_rl_repo", "/root/.axon_site/_ro/trn_rl_repo"):
    if os.path.isdir(_p) and _p not in sys.path:
        sys.path.insert(0, _p)

import concourse.bass as bass
import concourse.tile as tile
from concourse import bacc, mybir
from concourse.bass_utils import run_bass_kernel_spmd

F32 = mybir.dt.float32
F16 = mybir.dt.float16
AF = mybir.ActivationFunctionType
OP = mybir.AluOpType

B, S, D, F = 8, 2048, 1024, 3072
EPS = 1e-6
KD = D // 128          # 8 d-ptiles
MF2 = 2 * F // 128     # 48 f-ptiles (gate|up)
MFO = F // 128         # 24 f-ptiles
MD = D // 128          # 8 d-ptiles (output)

CH1 = 256              # phase-1 s-chunk
NCH1 = S // CH1
CH2 = 512              # phase-2 s-chunk
NCH2 = S // CH2
NST1 = CH1 // 128      # s-tiles per phase-1 chunk


def build_program():
    nc = bacc.Bacc("TRN2", target_bir_lowering=False, debug=False)

    x_d = nc.dram_tensor("x", [S, D], F32, kind="ExternalInput").ap()
    wmix_d = nc.dram_tensor("w_mix", [3 * MD, 128, KD, 128], F16, kind="ExternalInput").ap()
    bmix_d = nc.dram_tensor("b_mix", [128, 3 * MD], F32, kind="ExternalInput").ap()
    wgu_d = nc.dram_tensor("w_gu", [MF2, 128, KD, 128], F16, kind="ExternalInput").ap()
    wout_d = nc.dram_tensor("w_out", [MD, 128, MFO, 128], F16, kind="ExternalInput").ap()
    ident_d = nc.dram_tensor("ident", [128, 128], F32, kind="ExternalInput").ap()
    out_d = nc.dram_tensor("out", [S, D], F32, kind="ExternalOutput").ap()

    with tile.TileContext(nc) as tc, ExitStack() as top:
        # ---------- persistent tiles ----------
        cpool = top.enter_context(tc.tile_pool(name="consts", bufs=1))
        ident = cpool.tile([128, 128], F32)
        nc.sync.dma_start(ident[:], ident_d[:])
        ones_col = cpool.tile([128, 1], F16)
        nc.vector.memset(ones_col[:], 1.0)
        ones_row = cpool.tile([1, 128], F32)
        nc.vector.memset(ones_row[:], 1.0)
        zero128 = cpool.tile([128, 1], F32)
        nc.vector.memset(zero128[:], 0.0)
        eps1 = cpool.tile([1, 1], F32)
        nc.vector.memset(eps1[:], EPS)
        bmix = cpool.tile([128, 3 * MD], F32)
        nc.sync.dma_start(bmix[:], bmix_d[:])

        # DRAM scratch (tile-tracked so phase-2 reads order after phase-1
        # writes). One tile per phase-2 s-chunk so a phase-2 load only
        # depends on the phase-1 chunks that actually wrote it.
        dpool = top.enter_context(tc.tile_pool(name="dscratch", bufs=1, space="DRAM"))
        sc1_t = [dpool.tile([KD, 128, CH2], F32, name=f"sc1_{i}") for i in range(NCH2)]
        sc1_p = [t.rearrange("k p s -> p k s") for t in sc1_t]

        # normalized out1 stays resident in SBUF across phase 1 -> 2a
        o1n_pool = top.enter_context(tc.tile_pool(name="o1n", bufs=1))
        o1n = o1n_pool.tile([128, KD, S], F16)

        carry_pool = top.enter_context(tc.tile_pool(name="carry", bufs=1))
        carry = carry_pool.tile([128, KD], F32)

        # ---------- phase 1: mixer (software-pipelined over chunks) ----------
        with ExitStack() as ph1:
            wpool = ph1.enter_context(tc.tile_pool(name="wmix", bufs=1))
            wmix = wpool.tile([128, 3 * MD, KD, 128], F16)
            wmix_dp = wmix_d.rearrange("m p k j -> p m k j")

            p_nat = ph1.enter_context(tc.tile_pool(name="xnat", bufs=3))
            p_xT = ph1.enter_context(tc.tile_pool(name="xT", bufs=3))
            p_16 = ph1.enter_context(tc.tile_pool(name="f16bufs", bufs=2))
            p_sq = ph1.enter_context(tc.tile_pool(name="sqbufs", bufs=3))
            p_32 = ph1.enter_context(tc.tile_pool(name="f32bufs", bufs=2))
            p_row = ph1.enter_context(tc.tile_pool(name="rows", bufs=2))
            ps_tp = ph1.enter_context(tc.tile_pool(name="tp_ps", bufs=2, space="PSUM"))
            ps_mm = ph1.enter_context(tc.tile_pool(name="mm_ps", bufs=3, space="PSUM"))
            ps_ss = ph1.enter_context(tc.tile_pool(name="ss_ps", bufs=1, space="PSUM"))
            ps_bc = ph1.enter_context(tc.tile_pool(name="bc_ps", bufs=2, space="PSUM"))

            st_front = {}   # c -> (xT, rrow1)
            st_bc1 = {}     # c -> bc1 psum tile
            st_body = {}    # c -> (sig_g, tanh_v, sig_d)
            st_back = {}    # c -> (out1, rrow2)

            st_sq1 = {}

            def front_t(c):
                """load + transpose x chunk, squares (ACT)."""
                s0 = c * CH1
                xT = p_xT.tile([128, KD, CH1], F32, tag="xT", name=f"xT{c}")
                sq = p_sq.tile([128, KD, CH1], F16, tag="sq", name=f"sq1_{c}")
                nats = []
                for st in range(NST1):
                    xn_t = p_nat.tile([128, D], F32, tag="xnat", name=f"xnat{c}_{st}")
                    nc.sync.dma_start(xn_t[:], x_d[s0 + st * 128: s0 + (st + 1) * 128, :])
                    nats.append(xn_t)
                for kt in range(KD):
                    tp = ps_tp.tile([128, CH1], F32, tag="tp", name=f"tp{c}_{kt}")
                    for st in range(NST1):
                        nc.tensor.transpose(tp[:, st * 128:(st + 1) * 128],
                                            nats[st][:, kt * 128:(kt + 1) * 128],
                                            ident[:])
                    nc.scalar.copy(xT[:, kt], tp[:])
                    nc.scalar.activation(sq[:, kt], xT[:, kt], AF.Square,
                                         bias=zero128[:])
                st_front[c] = (xT, None)
                st_sq1[c] = sq

            def front_r(c):
                """norm1 reduce (PE) + sqrt/recip."""
                sq = st_sq1[c]
                ss = ps_ss.tile([1, CH1], F32, tag="ss", name=f"ss1_{c}")
                for kt in range(KD):
                    nc.tensor.matmul(ss[:], ones_col[:], sq[:, kt],
                                     start=(kt == 0), stop=(kt == KD - 1))
                srow = p_row.tile([1, CH1], F32, tag="srow1", name=f"srow1_{c}")
                nc.scalar.activation(srow[:], ss[:], AF.Sqrt, bias=eps1[:], scale=1.0 / D)
                rrow = p_row.tile([1, CH1], F32, tag="rrow1", name=f"rrow1_{c}")
                nc.vector.reciprocal(rrow[:], srow[:])
                st_front[c] = (st_front[c][0], rrow)

            def bcast1(c):
                rrow = st_front[c][1]
                bc = ps_bc.tile([128, CH1], F32, tag="bc", name=f"bc1_{c}")
                nc.tensor.matmul(bc[:], ones_row[:], rrow[:])
                st_bc1[c] = bc

            def body(c):
                """normalized input + g/v/d projections + activations."""
                xT = st_front[c][0]
                bc1 = st_bc1[c]
                xnT = p_16.tile([128, KD, CH1], F16, tag="xnT", name=f"xnT{c}")
                for kt in range(KD):
                    nc.vector.tensor_tensor(xnT[:, kt], xT[:, kt], bc1[:], OP.mult)
                sig_g = p_16.tile([128, KD, CH1], F16, tag="sig_g", name=f"sg{c}")
                tanh_v = p_16.tile([128, KD, CH1], F16, tag="tanh_v", name=f"tv{c}")
                sig_d = p_16.tile([128, KD, CH1], F16, tag="sig_d", name=f"sd{c}")
                for proj, (dst, fn) in enumerate(
                        ((sig_g, AF.Sigmoid), (tanh_v, AF.Tanh), (sig_d, AF.Sigmoid))):
                    for half in range(4):
                        ps = ps_mm.tile([128, 2, CH1], F32, tag="mm",
                                        name=f"mm{c}_{proj}_{half}")
                        for mi in range(2):
                            mt = proj * MD + half * 2 + mi
                            for kt in range(KD):
                                nc.tensor.matmul(ps[:, mi], wmix[:, mt, kt], xnT[:, kt],
                                                 start=(kt == 0), stop=(kt == KD - 1))
                        for mi in range(2):
                            mt = proj * MD + half * 2 + mi
                            nc.scalar.activation(dst[:, half * 2 + mi], ps[:, mi], fn,
                                                 bias=bmix[:, mt:mt + 1])
                st_body[c] = (sig_g, tanh_v, sig_d)

            def back_a(c):
                """scan inputs, scan, residual, norm2 squares+reduce.

                Per-kt pipeline so the norm2 PE reduce starts while later
                kt rows are still scanning on DVE."""
                sig_g, tanh_v, sig_d = st_body[c]
                xT = st_front[c][0]
                xs = p_16.tile([128, KD, CH1], F16, tag="xs", bufs=1, name=f"xs{c}")
                nc.vector.tensor_tensor(xs[:], sig_g[:], tanh_v[:], OP.mult)
                a_t = p_16.tile([128, KD, CH1], F16, tag="a_t", bufs=1, name=f"a{c}")
                nc.vector.tensor_scalar(a_t[:], sig_d[:], 0.998, 0.001, OP.mult, OP.add)
                hT = p_32.tile([128, KD, CH1], F32, tag="hT", bufs=1, name=f"hT{c}")
                out1 = p_32.tile([128, KD, CH1], F32, tag="out1", name=f"o1_{c}")
                sq = p_sq.tile([128, KD, CH1], F16, tag="sq", name=f"sq2_{c}")
                ss = ps_ss.tile([1, CH1], F32, tag="ss", name=f"ss2_{c}")
                for kt in range(KD):
                    init = 0.0 if c == 0 else carry[:, kt:kt + 1]
                    nc.vector.tensor_tensor_scan(hT[:, kt], a_t[:, kt], xs[:, kt],
                                                 init, OP.mult, OP.add)
                    nc.vector.tensor_copy(carry[:, kt:kt + 1], hT[:, kt, CH1 - 1:CH1])
                    nc.vector.tensor_tensor(out1[:, kt], xT[:, kt], hT[:, kt], OP.add)
                    nc.scalar.activation(sq[:, kt], out1[:, kt], AF.Square,
                                         bias=zero128[:])
                    nc.tensor.matmul(ss[:], ones_col[:], sq[:, kt],
                                     start=(kt == 0), stop=(kt == KD - 1))
                srow = p_row.tile([1, CH1], F32, tag="srow2", name=f"srow2_{c}")
                nc.scalar.activation(srow[:], ss[:], AF.Sqrt, bias=eps1[:], scale=1.0 / D)
                rrow = p_row.tile([1, CH1], F32, tag="rrow2", name=f"rrow2_{c}")
                nc.vector.reciprocal(rrow[:], srow[:])
                st_back[c] = (out1, rrow)

            def back_b(c):
                """norm2 broadcast, normalized out1 into resident SBUF, out1 bounce."""
                s0 = c * CH1
                out1, rrow = st_back[c]
                bc = ps_bc.tile([128, CH1], F32, tag="bc", name=f"bc2_{c}")
                nc.tensor.matmul(bc[:], ones_row[:], rrow[:])
                for kt in range(KD):
                    nc.vector.tensor_tensor(o1n[:, kt, s0:s0 + CH1], out1[:, kt],
                                            bc[:], OP.mult)
                sc, off = divmod(s0, CH2)
                nc.sync.dma_start(sc1_p[sc][:, :, off:off + CH1], out1[:])

            # pipelined emission; PE stream per cycle:
            #   [T(c+1)] [MM(c)] [R1(c+1)] [R2(c-1)] [B1(c+1)] [B2(c-1)]
            front_t(0)
            # mixer weights per e-ptile so the first projections start early
            for mt in range(3 * MD):
                nc.sync.dma_start(wmix[:, mt], wmix_dp[:, mt])
            front_r(0)
            bcast1(0)
            for c in range(NCH1):
                if c + 1 < NCH1:
                    front_t(c + 1)
                body(c)
                if c + 1 < NCH1:
                    front_r(c + 1)
                if c >= 1:
                    back_a(c - 1)
                if c + 1 < NCH1:
                    bcast1(c + 1)
                if c >= 1:
                    back_b(c - 1)
            back_a(NCH1 - 1)
            back_b(NCH1 - 1)

        # ---------- phase 2: FFN ----------
        with ExitStack() as ph2:
            zpool = ph2.enter_context(tc.tile_pool(name="zbuf", bufs=1))
            z = zpool.tile([128, MFO, S], F16)

            # 2a: gate/up + z
            with ExitStack() as ph2a:
                p_wgu = ph2a.enter_context(tc.tile_pool(name="wgu", bufs=4))
                p_gu = ph2a.enter_context(tc.tile_pool(name="gu16", bufs=3))
                ps_gu = ph2a.enter_context(tc.tile_pool(name="gu_ps", bufs=4, space="PSUM"))
                for mg in range(MFO):
                    wg = p_wgu.tile([128, KD, 128], F16, tag="wgu")
                    nc.sync.dma_start(wg[:], wgu_d[mg])
                    wu = p_wgu.tile([128, KD, 128], F16, tag="wgu")
                    nc.sync.dma_start(wu[:], wgu_d[MFO + mg])
                    for sc in range(NCH2):
                        sl = slice(sc * CH2, (sc + 1) * CH2)
                        gps = ps_gu.tile([128, CH2], F32, tag="gups")
                        for kt in range(KD):
                            nc.tensor.matmul(gps[:], wg[:, kt], o1n[:, kt, sl],
                                             start=(kt == 0), stop=(kt == KD - 1))
                        ups = ps_gu.tile([128, CH2], F32, tag="gups")
                        for kt in range(KD):
                            nc.tensor.matmul(ups[:], wu[:, kt], o1n[:, kt, sl],
                                             start=(kt == 0), stop=(kt == KD - 1))
                        sig = p_gu.tile([128, CH2], F16, tag="sig")
                        nc.scalar.activation(sig[:], gps[:], AF.Sigmoid, bias=zero128[:])
                        gate = p_gu.tile([128, CH2], F16, tag="gate")
                        nc.vector.tensor_tensor(gate[:], gps[:], sig[:], OP.mult)
                        up = p_gu.tile([128, CH2], F16, tag="up")
                        nc.scalar.copy(up[:], ups[:])
                        nc.vector.tensor_tensor(z[:, mg, sl], gate[:], up[:], OP.mult)

            # 2b: W_out + residual + transpose out
            with ExitStack() as ph2b:
                p_wo = ph2b.enter_context(tc.tile_pool(name="wout", bufs=3))
                p_o1c = ph2b.enter_context(tc.tile_pool(name="o1c", bufs=3))
                p_oT = ph2b.enter_context(tc.tile_pool(name="outT", bufs=MD + 1))
                p_onat = ph2b.enter_context(tc.tile_pool(name="onat", bufs=3))
                ps_y = ph2b.enter_context(tc.tile_pool(name="y_ps", bufs=2, space="PSUM"))
                ps_t2 = ph2b.enter_context(tc.tile_pool(name="t2_ps", bufs=2, space="PSUM"))
                for sc in range(NCH2):
                    sl = slice(sc * CH2, (sc + 1) * CH2)
                    outTs = []
                    for mo in range(MD):
                        wo = p_wo.tile([128, MFO, 128], F16, tag="wo",
                                       name=f"wo{sc}_{mo}")
                        nc.sync.dma_start(wo[:], wout_d[mo])
                        yps = ps_y.tile([128, CH2], F32, tag="yps")
                        for kt in range(MFO):
                            nc.tensor.matmul(yps[:], wo[:, kt], z[:, kt, sl],
                                             start=(kt == 0), stop=(kt == MFO - 1))
                        o1c = p_o1c.tile([128, CH2], F32, tag="o1c")
                        nc.sync.dma_start(o1c[:], sc1_t[sc][mo])
                        oT = p_oT.tile([128, CH2], F32, tag="oT")
                        nc.vector.tensor_tensor(oT[:], yps[:], o1c[:], OP.add)
                        outTs.append(oT)
                    for q in range(CH2 // 128):
                        onat = p_onat.tile([128, D], F32, tag="onat")
                        for h in range(2):
                            t2 = ps_t2.tile([128, 512], F32, tag="t2")
                            for j in range(4):
                                nc.tensor.transpose(
                                    t2[:, j * 128:(j + 1) * 128],
                                    outTs[4 * h + j][:, q * 128:(q + 1) * 128],
                                    ident[:])
                            nc.scalar.copy(onat[:, h * 512:(h + 1) * 512], t2[:])
                        srow0 = sc * CH2 + q * 128
                        nc.sync.dma_start(out_d[srow0:srow0 + 128, :], onat[:])

    nc.compile()
    return nc


_NC = None


def _get_nc():
    global _NC
    if _NC is None:
        _NC = build_program()
    return _NC


def _prep_weights(inputs):
    w1 = np.asarray(inputs["rms_mix_w"], np.float32)
    w2 = np.asarray(inputs["rms_ffn_w"], np.float32)
    Wg = np.asarray(inputs["Wg"], np.float32) * w1[None, :]
    Wv = np.asarray(inputs["Wv"], np.float32) * w1[None, :]
    Wd = np.asarray(inputs["Wd"], np.float32) * w1[None, :]
    Wcat = np.concatenate([Wg, Wv, Wd], axis=0)            # [3D, D]
    w_mix = np.ascontiguousarray(
        Wcat.T.reshape(KD, 128, 3 * MD, 128).transpose(2, 1, 0, 3)).astype(np.float16)
    bcat = np.concatenate([np.asarray(inputs["bg"], np.float32),
                           np.asarray(inputs["bv"], np.float32),
                           np.asarray(inputs["bd"], np.float32)])
    b_mix = np.ascontiguousarray(bcat.reshape(3 * MD, 128).T).astype(np.float32)
    Wgate = np.asarray(inputs["W_gate"], np.float32) * w2[None, :]
    Wup = np.asarray(inputs["W_up"], np.float32) * w2[None, :]
    Wcat2 = np.concatenate([Wgate, Wup], axis=0)           # [2F, D]
    w_gu = np.ascontiguousarray(
        Wcat2.T.reshape(KD, 128, MF2, 128).transpose(2, 1, 0, 3)).astype(np.float16)
    WoT = np.asarray(inputs["W_out"], np.float32).T        # [F, D]
    w_out = np.ascontiguousarray(
        WoT.reshape(MFO, 128, MD, 128).transpose(2, 1, 0, 3)).astype(np.float16)
    return {
        "w_mix": w_mix, "b_mix": b_mix, "w_gu": w_gu, "w_out": w_out,
        "ident": np.eye(128, dtype=np.float32),
    }


def run(inputs, trace=False, **kw):
    x = np.asarray(inputs["x"], np.float32)
    shared = _prep_weights(inputs)
    in_maps = [dict(shared, x=np.ascontiguousarray(x[b])) for b in range(B)]
    res = run_bass_kernel_spmd(_get_nc(), in_maps, list(range(B)), trace=trace, **kw)
    out = np.stack([np.asarray(res.results[b]["out"], np.float32) for b in range(B)])
    return out, res


def kernel(**inputs) -> np.ndarray:
    out, _ = run(inputs)
    return out
